# revision 73
# baseline (speedup 1.0000x reference)
"""Trainium2 Bass kernel for nn_ArcDecoderLayer (sparse_attention).

Self-contained: takes FULL unsharded inputs, shards across 8 NeuronCores
(head-parallel attention + FF-parallel MLP), returns the FULL output.

v2 restructure vs baseline:
 - o_proj is computed locally per-core (contraction over the core's 4 heads)
   and combined with a per-block AllReduce (with hidden/8 folded in) that
   directly yields h = hidden + attn_out on every core.  This replaces the
   attn AllGather + gathered o_proj + o AllGather chain.
 - The MLP down projection is computed locally (contraction over the core's
   1024 FF dims, output = full D) and combined with a ReduceScatter per
   S-half (with h/8 folded in) that directly yields the final output slice.
   This replaces the FF-intermediate AllGather + gathered down proj.
 - Attention, o_proj+AllReduce, LN2 and the MLP are pipelined per 512-wide
   sequence block, so collectives overlap compute.
 - Softmax denominator / self-key vector work runs on full 128-partition
   tiles instead of single-row slices; LN squares run on DVE instead of
   the scalar engine; SiLU uses the fused Silu activation.
 - wg/wu weights are streamed per quarter per block to fit SBUF alongside
   the attention working set.
"""

import sys
import types

sys.path.insert(0, "/opt/trn_rl_repo")

# ---- shim antenv.axon_hooks so trace=True profiling works in this image ----
if "antenv.axon_hooks" not in sys.modules:
    _hook_mod = types.ModuleType("antenv.axon_hooks")
    _hook_state = {"hook": None}

    def _set_hook(h):
        _hook_state["hook"] = h

    def _get_hook():
        return _hook_state["hook"]

    _hook_mod.set_axon_ntff_profile_hook = _set_hook
    _hook_mod.get_axon_ntff_profile_hook = _get_hook
    sys.modules["antenv.axon_hooks"] = _hook_mod
    try:
        import antenv

        antenv.axon_hooks = _hook_mod
        from trn_agent_boot.trn_boot import _ntff_profile_via_ctypes

        _set_hook(_ntff_profile_via_ctypes("/opt/axon/libaxon_pjrt.so"))
    except Exception:
        pass

import numpy as np
import ml_dtypes

import concourse.bass as bass
import concourse.mybir as mybir
import concourse.tile as tile
from concourse import library_config
from concourse.vector_clock import ScopedClock

BF16 = ml_dtypes.bfloat16

N_CORES = 8
D = 2048
FF = 8192
H = 32
DH = 64
RD = 16
EPS = 1e-5
BASE = 10000.0

J = D // N_CORES        # 256 head-dims per core (4 heads)
FFL = FF // N_CORES     # 1024 ff dims per core
KC = D // 128           # 16 contraction chunks
NBLK = 512              # seq block width
MD = J // 128           # 2 Mtiles in the core's J slice
FC = FFL // 128         # 8 contraction chunks for the down proj
MF = FFL // 128         # 8 output Mtiles for g/u


WAIT_LIMITS = {"InstNoOp": 1, "InstDrain": 1, "InstEventSemaphore": 1}
DEFAULT_WAIT_LIMIT = 1

DEBUG = False


class PatchedTC(tile.TileContext):
    """TileContext patched for this walrus build, which rejects instructions
    carrying more than a couple of sync wait commands: excess waits are
    split onto injected same-engine nops just before the instruction."""

    _wsplit_n = 0

    def _split_excess_waits(self, ordered):
        for bb, insts in ordered.items():
            out = []
            for inst in insts:
                si = inst.sync_info
                waits = list(si.on_wait) if si and si.on_wait else []
                lim = WAIT_LIMITS.get(type(inst).__name__,
                                      DEFAULT_WAIT_LIMIT)
                if len(waits) > lim:
                    for w in waits[:-lim]:
                        nop = mybir.InstNoOp(
                            name=f"I-wsplit-{PatchedTC._wsplit_n}",
                            ins=[], outs=[], engine=inst.engine,
                            nofuse=True)
                        PatchedTC._wsplit_n += 1
                        nop.sync_info = mybir.SyncInfo(
                            on_wait=[w], on_update=[])
                        out.append(nop)
                    inst.sync_info = mybir.SyncInfo(
                        on_wait=waits[-lim:],
                        on_update=list(si.on_update or []))
                out.append(inst)
            ordered[bb] = out

    def _lower_ordered_insts(self, ordered):
        self._split_excess_waits(ordered)
        return super()._lower_ordered_insts(ordered)

    def _drain_and_barrier(self, tick_clock, wait_clock):
        nc = self.nc
        probe = nc.sync.nop(nofuse=True, hint="tail_wait_probe")
        wait_clock.add_sem_waits(
            probe.ins, ScopedClock({None: tick_clock.global_clock})
        )
        waits = list(probe.ins.sync_info.on_wait or [])
        probe.ins.sync_info.on_wait = waits[:1]
        for i in range(1, len(waits)):
            n = nc.sync.nop(nofuse=True, hint=f"tail_wait_{i}")
            n.ins.sync_info = mybir.SyncInfo(on_wait=[waits[i]], on_update=[])
        nc.sync.drain()
        nc.all_engine_barrier()
        assert self.sems is not None
        popped = nc._tile_sem_poison_stack.pop()
        assert popped is self._sem_poison
        nc.clear_and_free_semaphores(list(self.sems.allocated().values()))
        nc.all_engine_barrier()


def build_graph(S):
    """Build the SPMD 8-core graph for sequence length S (multiple of 512)."""
    dt = mybir.dt
    f32, bf16 = dt.float32, dt.bfloat16
    AF = mybir.ActivationFunctionType
    Alu = mybir.AluOpType
    NB = S // NBLK          # seq blocks
    LT = S // 128           # 128-wide l tiles
    HF = NB // 2            # ReduceScatter halves

    nc = bass.Bass()
    P = nc.declare_dram_parameter

    xm_e = P("xm", [128, KC, S], bf16, isOutput=False)
    xh_e = P("xh", [128, KC, S], bf16, isOutput=False)
    wq_e = P("wq", [128, KC, J], bf16, isOutput=False)
    wk_e = P("wk", [128, KC, J], bf16, isOutput=False)
    wv_e = P("wv", [128, KC, J], bf16, isOutput=False)
    woT_e = P("woT", [128, MD, D], bf16, isOutput=False)
    wg_e = P("wg", [128, KC, FFL], bf16, isOutput=False)
    wu_e = P("wu", [128, KC, FFL], bf16, isOutput=False)
    wdT_e = P("wdT", [128, FC, D], bf16, isOutput=False)
    # column (per-partition) weight rowsums + biases for q/k/vTh epilogues
    wsq_e = P("wsq", [128, 2], f32, isOutput=False)
    wsk_e = P("wsk", [128, 2], f32, isOutput=False)
    wsvc_e = P("wsvc", [128, 2], f32, isOutput=False)
    bq_e = P("bq", [128, 2], f32, isOutput=False)
    bk_e = P("bk", [128, 2], f32, isOutput=False)
    bvc_e = P("bvc", [128, 2], f32, isOutput=False)
    # row layouts for v_mem epilogue
    wsv_e = P("wsv_row", [1, J], f32, isOutput=False)
    bv_e = P("bv_row", [1, J], f32, isOutput=False)
    bg_e = P("bg", [128, MF], f32, isOutput=False)
    bu_e = P("bu", [128, MF], f32, isOutput=False)
    ropec_e = P("rope_cos", [128, S], bf16, isOutput=False)
    ropes_e = P("rope_sinsg", [128, S], bf16, isOutput=False)
    masks_e = P("masks", [128, 4, NBLK], bf16, isOutput=False)
    out_e = P("out", [J, S], bf16, isOutput=True)
    hdbg_e = P("hdbg", [D, S], bf16, isOutput=True) if DEBUG else None

    rg = [list(range(N_CORES))]

    with PatchedTC(nc) as tc:
        with (
            tc.tile_pool(name="const", bufs=1) as constp,
            tc.tile_pool(name="dram", bufs=1, space="DRAM") as dramp,
            tc.tile_pool(name="dsh", bufs=1, space="DRAM") as dshp,
        ):
            kqvp = tc.alloc_tile_pool(name="kqv", bufs=1)
            statkp = tc.alloc_tile_pool(name="statk", bufs=1)

            masks_t = constp.tile([128, 4, NBLK], bf16)
            nc.sync.dma_start(masks_t[:], masks_e[:])
            # tiny warm-up AllReduce: pays the first-collective fixed cost
            # during phase M instead of on the critical AR_0
            warm_in = dramp.tile([128, 8], bf16, name="warm_in")
            warm_out = dshp.tile([128, 8], bf16, name="warm_out",
                                 addr_space="Shared")
            warm_sb = constp.tile([128, 8], bf16)
            nc.vector.memset(warm_sb[:], 0.0)
            nc.scalar.dma_start(warm_in[:], warm_sb[:])
            nc.gpsimd.collective_compute(
                "AllReduce", mybir.AluOpType.add,
                replica_groups=[list(range(N_CORES))],
                ins=[warm_in.opt()], outs=[warm_out.opt()])
            ones_c = constp.tile([128, 1], bf16)
            nc.vector.memset(ones_c[:], 1.0)
            ones128 = constp.tile([128, 128], bf16)
            nc.vector.memset(ones128[:], 1.0)
            eps_c = constp.tile([128, 1], f32)
            nc.vector.memset(eps_c[:], EPS)
            onesf = constp.tile([1, 128], f32)
            nc.vector.memset(onesf[:], 1.0)

            wsq_t = constp.tile([128, 2], f32)
            nc.sync.dma_start(wsq_t[:], wsq_e[:])
            wsk_t = constp.tile([128, 2], f32)
            nc.sync.dma_start(wsk_t[:], wsk_e[:])
            wsvc_t = constp.tile([128, 2], f32)
            nc.sync.dma_start(wsvc_t[:], wsvc_e[:])
            bq_t = constp.tile([128, 2], f32)
            nc.sync.dma_start(bq_t[:], bq_e[:])
            bk_t = constp.tile([128, 2], f32)
            nc.sync.dma_start(bk_t[:], bk_e[:])
            bvc_t = constp.tile([128, 2], f32)
            nc.sync.dma_start(bvc_t[:], bvc_e[:])
            bg_t = constp.tile([128, MF], f32)
            nc.sync.dma_start(bg_t[:], bg_e[:])
            bu_t = constp.tile([128, MF], f32)
            nc.sync.dma_start(bu_t[:], bu_e[:])

            def bcast_rows(dst, src_row, width, pspool, ones_row):
                """dst[0:128, :width] = src_row[0, :width] via K=1 matmuls."""
                for i in range(0, width, NBLK):
                    w = min(NBLK, width - i)
                    ps = pspool.tile([128, NBLK], f32, name="bc_ps",
                                     tag="bc_ps", bufs=1)
                    nc.tensor.matmul(ps[:, :w], ones_row[0:1, :],
                                     src_row[0:1, i:i + w],
                                     start=True, stop=True)
                    nc.vector.tensor_copy(dst[:, i:i + w], ps[:, :w])

            # persistent QKV outputs (mem-part k only; self-keys are folded
            # into the qk product and never stored)
            kT = [kqvp.tile([128, S], bf16, name=f"kT{m}") for m in range(2)]
            qT = [kqvp.tile([128, S], bf16, name=f"qT{m}") for m in range(2)]
            vTh = [kqvp.tile([128, S], bf16, name=f"vTh{m}") for m in range(2)]
            v_mem = kqvp.tile([128, LT, J], bf16)
            sf_all = kqvp.tile([128, S], f32)   # self-key raw scores, rows
            #                                     {0,32,64,96} valid

            # v_mem epilogue needs column-layout stats of the mem part
            rstd_col_mem = statkp.tile([128, LT], f32)
            c_col_mem = statkp.tile([128, LT], f32)

            # ---------- LN1 stats + QKV -----------------------------------
            def ln_stats(xpart, sqp, psp, smallp, rowp, part_name,
                         want_col):
                """Returns (rstd_col, c_col, rstd_b, c_b) for one x part.

                Stats are over the 128*KC feature dim per l column.
                Sum is accumulated column-major (N=1 matmuls); sumsq
                row-major (squares split scalar/DVE, ones as lhsT).
                """
                sum_ps = psp.tile([128, LT], f32, name="sum_ps",
                                  tag="sum_ps")
                for lt in range(LT):
                    sl = slice(lt * 128, (lt + 1) * 128)
                    for kc in range(KC):
                        nc.tensor.matmul(
                            sum_ps[:, lt:lt + 1],
                            xpart[:, kc, sl], ones_c[:],
                            start=(kc == 0), stop=(kc == KC - 1))
                sumsq_row = rowp.tile([1, S], f32, name="sumsq_row",
                                      tag="strow")
                for nb in range(NB):
                    sq_ps = psp.tile([1, NBLK], f32, name="sq_ps",
                                     tag="sq_ps")
                    for kc in range(KC):
                        sq_t = sqp.tile([128, NBLK], bf16, name="sq_t")
                        xs = xpart[:, kc, nb * NBLK:(nb + 1) * NBLK]
                        if kc % 2 == 0:
                            nc.scalar.activation(sq_t[:], xs, AF.Square)
                        else:
                            nc.vector.tensor_mul(sq_t[:], xs, xs)
                        nc.tensor.matmul(
                            sq_ps[:], ones_c[:], sq_t[:],
                            start=(kc == 0), stop=(kc == KC - 1))
                    nc.vector.tensor_copy(
                        sumsq_row[:, nb * NBLK:(nb + 1) * NBLK], sq_ps[:])
                # sumsq row -> col via DRAM
                drq = dramp.tile([S], f32, name=f"st_sq_{part_name}")
                nc.gpsimd.dma_start(
                    drq[:].rearrange("(o a) -> o a", o=1), sumsq_row[:])
                sumsq_col = smallp.tile([128, LT], f32, name="sumsq_col")
                nc.gpsimd.dma_start(
                    sumsq_col[:], drq[:].rearrange("(t p) -> p t", p=128))
                mean_c = smallp.tile([128, LT], f32, name="mean_c")
                nc.vector.tensor_scalar_mul(mean_c[:], sum_ps[:], 1.0 / D)
                ex2_c = smallp.tile([128, LT], f32, name="ex2_c")
                nc.vector.tensor_scalar_mul(ex2_c[:], sumsq_col[:], 1.0 / D)
                m2_c = smallp.tile([128, LT], f32, name="m2_c")
                nc.vector.tensor_mul(m2_c[:], mean_c[:], mean_c[:])
                var_c = smallp.tile([128, LT], f32, name="var_c")
                nc.vector.tensor_sub(var_c[:], ex2_c[:], m2_c[:])
                sd_c = smallp.tile([128, LT], f32, name="sd_c")
                nc.scalar.activation(sd_c[:], var_c[:], AF.Sqrt, bias=eps_c[:])
                rstd_c = smallp.tile([128, LT], f32, name="rstd_c")
                nc.vector.reciprocal(rstd_c[:], sd_c[:])
                c_c = smallp.tile([128, LT], f32, name="c_c")
                nc.vector.tensor_mul(c_c[:], mean_c[:], rstd_c[:])
                # col -> row roundtrip through DRAM, then partition-broadcast
                outs = []
                for nm, col in (("rstd", rstd_c), ("c", c_c)):
                    dr = dramp.tile([S], f32, name=f"st_{nm}_{part_name}")
                    nc.gpsimd.dma_start(
                        dr[:].rearrange("(t p) -> p t", p=128), col[:])
                    row = rowp.tile([1, S], f32, name=f"row_{nm}",
                                    tag="strow")
                    nc.gpsimd.dma_start(
                        row[:], dr[:].rearrange("(o a) -> o a", o=1))
                    row16 = rowp.tile([1, S], bf16, name=f"row16_{nm}",
                                      tag="strow16")
                    nc.vector.tensor_copy(row16[:], row[:])
                    bcast = rowp.tile([128, S], bf16, name=f"bc_{nm}")
                    bcast_rows(bcast, row16, S, psp, ones128)
                    outs.append(bcast)
                return rstd_c, c_c, outs[0], outs[1]

            def proj_rows(wt, dst, xpart, rstd_b, c_b, ws_t, b_t, psp,
                          cwp):
                """q/k/vTh-style projection. Raw matmul results are copied
                to dst immediately; the LN epilogue is applied in-place
                after stats are ready (cw blocks computed lazily)."""
                for m in range(2):
                    for nb in range(NB):
                        ps = psp.tile([128, NBLK], f32, name="proj_ps",
                                      tag="proj_ps", bufs=2)
                        bsl = slice(nb * NBLK, (nb + 1) * NBLK)
                        for kc in range(KC):
                            nc.tensor.matmul(
                                ps[:],
                                wt[:, kc, m * 128:(m + 1) * 128],
                                xpart[:, kc, bsl],
                                start=(kc == 0), stop=(kc == KC - 1))
                        d = dst[m][:, bsl]
                        nc.scalar.activation(d, ps[:], AF.Copy)
                for m in range(2):
                    for nb in range(NB):
                        sl = slice(nb * NBLK, (nb + 1) * NBLK)
                        cw = cwp.tile([128, NBLK], bf16, name="cw_blk",
                                      tag="cw_blk", bufs=2)
                        nc.vector.tensor_scalar(
                            out=cw[:], in0=c_b[:, sl],
                            scalar1=ws_t[:, m:m + 1],
                            scalar2=b_t[:, m:m + 1],
                            op0=Alu.mult, op1=Alu.subtract)
                        d = dst[m][:, sl]
                        nc.vector.tensor_mul(d, d, rstd_b[:, sl])
                        nc.vector.tensor_sub(d, d, cw[:])

            with (
                tc.tile_pool(name="wqkv", bufs=1) as wqkvp,
                tc.tile_pool(name="psq", bufs=1, space="PSUM") as psqp,
                tc.tile_pool(name="psst", bufs=1, space="PSUM") as psstp,
            ):
                wq_t = wqkvp.tile([128, KC, J], bf16)
                nc.sync.dma_start(wq_t[:], wq_e[:])
                wk_t = wqkvp.tile([128, KC, J], bf16)
                nc.sync.dma_start(wk_t[:], wk_e[:])
                wv_t = wqkvp.tile([128, KC, J], bf16)
                nc.sync.dma_start(wv_t[:], wv_e[:])

                ropecp = tc.alloc_tile_pool(name="ropec", bufs=1)
                cos_t = ropecp.tile([128, S], bf16)
                nc.sync.dma_start(cos_t[:], ropec_e[:])
                sin_t = ropecp.tile([128, S], bf16)
                nc.sync.dma_start(sin_t[:], ropes_e[:])
                xmp = tc.alloc_tile_pool(name="xm", bufs=1)
                xm_t = xmp.tile([128, KC, S], bf16)
                for kc in range(KC):
                    eng = nc.sync if kc % 2 == 0 else nc.scalar
                    eng.dma_start(xm_t[:, kc, :], xm_e[:, kc, :])

                def rope(dst_tiles, ropep):
                    for m in range(2):
                        t = dst_tiles[m]
                        for o in (0, 64):
                            sw = ropep.tile([128, S], bf16,
                                            name="rope_sw", bufs=1)
                            nc.gpsimd.dma_start(
                                sw[o:o + 8, :], t[o + 8:o + 16, :])
                            nc.gpsimd.dma_start(
                                sw[o + 8:o + 16, :], t[o:o + 8, :])
                            tc_ = ropep.tile([128, S], bf16,
                                             name="rope_tc", bufs=1)
                            nc.vector.tensor_mul(
                                tc_[o:o + 16, :], t[o:o + 16, :],
                                cos_t[o:o + 16, :])
                            nc.vector.tensor_mul(
                                sw[o:o + 16, :], sw[o:o + 16, :],
                                sin_t[o:o + 16, :])
                            nc.vector.tensor_add(
                                t[o:o + 16, :], tc_[o:o + 16, :],
                                sw[o:o + 16, :])

                # ----- phase M: memory part -----
                with (
                    tc.tile_pool(name="sqa", bufs=2) as sqap,
                    tc.tile_pool(name="sma", bufs=1) as smap,
                    tc.tile_pool(name="rowa", bufs=1) as rowap,
                ):
                    rs_c, c_c, rstd_bm, c_bm = ln_stats(
                        xm_t, sqap, psstp, smap, rowap, "mem", True)
                    nc.vector.tensor_copy(rstd_col_mem[:], rs_c[:])
                    nc.vector.tensor_copy(c_col_mem[:], c_c[:])
                    wsvb = smap.tile([128, J], f32)
                    wsv_row = smap.tile([1, J], f32)
                    nc.sync.dma_start(wsv_row[:], wsv_e[:])
                    bvb = smap.tile([128, J], f32)
                    bv_row = smap.tile([1, J], f32)
                    nc.sync.dma_start(bv_row[:], bv_e[:])
                    bcast_rows(wsvb, wsv_row, J, psstp, onesf)
                    bcast_rows(bvb, bv_row, J, psstp, onesf)
                    proj_rows(wk_t, kT, xm_t, rstd_bm, c_bm, wsk_t, bk_t,
                              psqp, sqap)
                    # v_mem row-major: lhsT = xm l-tile, rhs = wv
                    for lt in range(LT):
                        ps = psqp.tile([128, J], f32, name="vm_ps",
                                       tag="vm_ps", bufs=2)
                        for kc in range(KC):
                            nc.tensor.matmul(
                                ps[:],
                                xm_t[:, kc, lt * 128:(lt + 1) * 128],
                                wv_t[:, kc, :],
                                start=(kc == 0), stop=(kc == KC - 1))
                        nc.vector.tensor_copy(v_mem[:, lt, :], ps[:])
                    for lt in range(LT):
                        # cwv = c*wsv - bv in one fused op
                        cwv = sqap.tile([128, J], f32, name="cwv")
                        nc.vector.scalar_tensor_tensor(
                            out=cwv[:], in0=wsvb[:],
                            scalar=c_col_mem[:, lt:lt + 1], in1=bvb[:],
                            op0=Alu.mult, op1=Alu.subtract)
                        nc.vector.tensor_scalar_mul(
                            v_mem[:, lt, :], v_mem[:, lt, :],
                            rstd_col_mem[:, lt:lt + 1])
                        nc.vector.tensor_sub(
                            v_mem[:, lt, :], v_mem[:, lt, :], cwv[:])
                    # rope the memory keys here so it overlaps phase H
                    rope(kT, sqap)
                xmp.release()

                # ----- phase H: hidden part -----
                with (
                    tc.tile_pool(name="xh", bufs=1) as xhp,
                    tc.tile_pool(name="sqb", bufs=2) as sqbp,
                    tc.tile_pool(name="smb", bufs=1) as smbp,
                    tc.tile_pool(name="rowb", bufs=1) as rowbp,
                    tc.tile_pool(name="khp", bufs=1) as khp,
                ):
                    xh_t = xhp.tile([128, KC, S], bf16)
                    for kc in range(KC):
                        eng = nc.scalar if kc % 2 == 0 else nc.sync
                        eng.dma_start(xh_t[:, kc, :], xh_e[:, kc, :])
                    _, _, rstd_bh, c_bh = ln_stats(
                        xh_t, sqbp, psstp, smbp, rowbp, "hid", False)
                    kh = [khp.tile([128, S], bf16, name=f"kh{m}")
                          for m in range(2)]
                    # kh first, then q: the qk self product needs pre-rope
                    # q, and roping q right after its epilogue unblocks
                    # the attention S-matmuls while vTh still projects.
                    proj_rows(wk_t, kh, xh_t, rstd_bh, c_bh, wsk_t, bk_t,
                              psqp, sqbp)
                    proj_rows(wq_t, qT, xh_t, rstd_bh, c_bh, wsq_t, bq_t,
                              psqp, sqbp)

                    # self-key raw scores BEFORE RoPE (equal positions =>
                    # rotation preserves the dot product): sf_all rows
                    # {0,32,64,96} = sum over head dims of q*k_self
                    for m in range(2):
                        nc.vector.tensor_mul(kh[m][:], qT[m][:], kh[m][:])
                    rope(qT, sqbp)
                    for b in range(NB):
                        bsl = slice(b * NBLK, (b + 1) * NBLK)
                        sf_ps = psqp.tile([128, NBLK], f32, name="sf_ps",
                                          tag="proj_ps", bufs=2)
                        for m in range(2):
                            for o in (0, 64):
                                hsl = slice(o, o + 64)
                                r = 32 * (2 * m + o // 64)
                                nc.tensor.matmul(
                                    sf_ps[r:r + 1, :], ones_c[hsl, 0:1],
                                    kh[m][hsl, bsl],
                                    start=True, stop=True,
                                    tile_position=(o, r))
                        nc.scalar.activation(sf_all[:, bsl], sf_ps[:],
                                             AF.Copy)
                    proj_rows(wv_t, vTh, xh_t, rstd_bh, c_bh, wsvc_t,
                              bvc_t, psqp, sqbp)
                ropecp.release()

            # ---------- phase A: attention + local o_proj + AllReduce -----
            o_bnc = [dramp.tile([D, NBLK], bf16, name=f"o_bnc{b}")
                     for b in range(NB)]
            h_sh = [dshp.tile([D, NBLK], bf16, name=f"h_sh{b}",
                              addr_space="Shared") for b in range(NB)]
            d_bnc = [dramp.tile([D, NBLK], bf16, name=f"d_bnc{b}")
                     for b in range(NB)]
            fin = [dramp.tile([J, NBLK], bf16, name=f"fin{b}")
                   for b in range(NB)]
            # the last block's RS is split in two interleaved halves so
            # only ~half of it is exposed after the final down proj.
            # d3h[i] chunk c holds D rows c*256 + i*128 .. + 127.
            d3h = [dramp.tile([D // 2, NBLK], bf16, name=f"d3h{i}")
                   for i in range(2)]
            fin3 = [dramp.tile([J // 2, NBLK], bf16, name=f"fin3{i}")
                    for i in range(2)]

            wmatp = tc.alloc_tile_pool(name="wmats", bufs=1)
            woT_t = wmatp.tile([128, MD, D], bf16)
            nc.scalar.dma_start(woT_t[:], woT_e[:])
            wdT_t = wmatp.tile([128, FC, D], bf16)
            nc.scalar.dma_start(wdT_t[:], wdT_e[:])

            attwp = tc.alloc_tile_pool(name="attw", bufs=1)
            atttp = tc.alloc_tile_pool(name="attt", bufs=1)
            attrp = tc.alloc_tile_pool(name="attr", bufs=1)
            xhbp = tc.alloc_tile_pool(name="xhb", bufs=4)
            mlpwp = tc.alloc_tile_pool(name="mlpw", bufs=2)
            hbp = tc.alloc_tile_pool(name="hbp", bufs=1)
            hdp = tc.alloc_tile_pool(name="hdp", bufs=2)
            h2p = tc.alloc_tile_pool(name="h2p", bufs=1)
            mlocp = tc.alloc_tile_pool(name="mlocp", bufs=2)
            gutp = tc.alloc_tile_pool(name="gut", bufs=2)
            sq2p = tc.alloc_tile_pool(name="sq2", bufs=2)
            sm2p = tc.alloc_tile_pool(name="sm2", bufs=1)
            psSp = tc.alloc_tile_pool(name="psS", bufs=2, space="PSUM")
            psAp = tc.alloc_tile_pool(name="psA", bufs=1, space="PSUM")
            psDnp = tc.alloc_tile_pool(name="psDen", bufs=1, space="PSUM")
            psBrp = tc.alloc_tile_pool(name="psBr", bufs=1, space="PSUM")
            psGp = tc.alloc_tile_pool(name="psG", bufs=1, space="PSUM")
            psUp = tc.alloc_tile_pool(name="psU", bufs=1, space="PSUM")
            psD2p = tc.alloc_tile_pool(name="psD2", bufs=1, space="PSUM")
            if True:
                def attention_block(b):
                    bsl = slice(b * NBLK, (b + 1) * NBLK)
                    den4 = psDnp.tile([128, NBLK], f32, name="den4")
                    avs = []
                    for m in range(2):
                        ap_ps = psAp.tile([128, NBLK], f32, name="ap")
                        for o in (0, 64):
                            hsl = slice(o, o + 64)
                            r = 32 * (2 * m + o // 64)
                            rsl = slice(r, r + 1)
                            for t in range(4 * b + 4):
                                s_ps = psSp.tile([128, NBLK], f32,
                                                 name="s_ps", tag="smm")
                                nc.tensor.matmul(
                                    s_ps[:],
                                    kT[m][hsl, t * 128:(t + 1) * 128],
                                    qT[m][hsl, bsl],
                                    start=True, stop=True,
                                    tile_position=(o, 0))
                                w_t = attwp.tile([128, NBLK], bf16,
                                                 name="w_t", bufs=3)
                                nc.scalar.activation(
                                    w_t[:], s_ps[:], AF.Exp, scale=0.125)
                                if t >= 4 * b:
                                    nc.vector.tensor_mul(
                                        w_t[:], w_t[:],
                                        masks_t[:, t - 4 * b, :])
                                nc.tensor.matmul(
                                    ap_ps[hsl, :],
                                    v_mem[:, t, m * 128 + o:
                                          m * 128 + o + 64],
                                    w_t[:],
                                    start=(t == 0), stop=(t == 4 * b + 3),
                                    tile_position=(0, o))
                                nc.tensor.matmul(
                                    den4[rsl, :], ones_c[:, 0:1], w_t[:],
                                    start=(t == 0), stop=(t == 4 * b + 3),
                                    tile_position=(0, r))
                        av_sb = attwp.tile([128, NBLK], bf16,
                                           name=f"av_sb{m}", bufs=1)
                        nc.vector.tensor_copy(av_sb[:], ap_ps[:])
                        avs.append(av_sb)
                    # widened self/denominator chain (rows {0,32,64,96}
                    # meaningful, other rows harmless garbage)
                    swf = attrp.tile([128, NBLK], f32, name="swf")
                    nc.scalar.activation(swf[:], sf_all[:, bsl], AF.Exp,
                                         scale=0.125)
                    dent = attrp.tile([128, NBLK], f32, name="dent")
                    nc.vector.tensor_add(dent[:], den4[:], swf[:])
                    rcp = attrp.tile([128, NBLK], f32, name="rcp")
                    nc.vector.reciprocal(rcp[:], dent[:])
                    swb = attrp.tile([128, NBLK], bf16, name="swb")
                    nc.vector.tensor_copy(swb[:], swf[:])
                    rcpb = attrp.tile([128, NBLK], bf16, name="rcpb")
                    nc.vector.tensor_copy(rcpb[:], rcp[:])
                    cmbs = []
                    for m in range(2):
                        sb_ps = psBrp.tile([128, NBLK], f32, name="br",
                                           tag="br")
                        for o in (0, 64):
                            r = 32 * (2 * m + o // 64)
                            rsl = slice(r, r + 1)
                            nc.tensor.matmul(
                                sb_ps[o:o + 64, :], ones128[rsl, 0:64],
                                swb[rsl, :], start=True, stop=True,
                                tile_position=(r, o))
                        t0 = atttp.tile([128, NBLK], bf16, name="t0",
                                        bufs=2)
                        nc.vector.tensor_mul(t0[:], vTh[m][:, bsl],
                                             sb_ps[:])
                        rb_ps = psBrp.tile([128, NBLK], f32, name="br",
                                           tag="br")
                        for o in (0, 64):
                            r = 32 * (2 * m + o // 64)
                            rsl = slice(r, r + 1)
                            nc.tensor.matmul(
                                rb_ps[o:o + 64, :], ones128[rsl, 0:64],
                                rcpb[rsl, :], start=True, stop=True,
                                tile_position=(r, o))
                        t1 = atttp.tile([128, NBLK], bf16, name="t1",
                                        bufs=2)
                        nc.vector.tensor_add(t1[:], avs[m][:], t0[:])
                        cmb = atttp.tile([128, NBLK], bf16, name=f"cmb{m}",
                                         bufs=1)
                        nc.vector.tensor_mul(cmb[:], t1[:], rb_ps[:])
                        cmbs.append(cmb)
                    # local o_proj: out = full D, contraction over local J;
                    # fold hidden/8 so the AllReduce yields h directly
                    for md in range(KC):
                        o_ps = psSp.tile([128, NBLK], f32, name="o_ps",
                                         tag="smm")
                        nc.tensor.matmul(
                            o_ps[:], woT_t[:, 0, md * 128:(md + 1) * 128],
                            cmbs[0][:], start=True, stop=False)
                        nc.tensor.matmul(
                            o_ps[:], woT_t[:, 1, md * 128:(md + 1) * 128],
                            cmbs[1][:], start=False, stop=True)
                        xhb = xhbp.tile([128, NBLK], bf16, name="xhb")
                        nc.sync.dma_start(xhb[:], xh_e[:, md, bsl])
                        oc = atttp.tile([128, NBLK], bf16, name="oc",
                                        bufs=2)
                        nc.vector.scalar_tensor_tensor(
                            out=oc[:], in0=xhb[:], scalar=0.125,
                            in1=o_ps[:], op0=Alu.mult, op1=Alu.add)
                        nc.scalar.dma_start(
                            o_bnc[b][md * 128:(md + 1) * 128, :], oc[:])
                    nc.gpsimd.collective_compute(
                        "AllReduce", Alu.add, replica_groups=rg,
                        ins=[o_bnc[b].opt()], outs=[h_sh[b].opt()])

                def emit_down(bb, md, hb_t, m_loc, pool, tag,
                              dst_ap=None):
                    d_ps = pool.tile([128, NBLK], f32, name="d_ps",
                                     tag=tag)
                    for fc in range(FC):
                        nc.tensor.matmul(
                            d_ps[:], wdT_t[:, fc, md * 128:(md + 1) * 128],
                            m_loc[:, fc, :],
                            start=(fc == 0), stop=(fc == FC - 1))
                    hd = hdp.tile([128, NBLK], bf16, name="hd")
                    nc.sync.dma_start(
                        hd[:],
                        h_sh[bb][md * 128:(md + 1) * 128, :])
                    db = gutp.tile([128, NBLK], bf16, name="db")
                    nc.vector.scalar_tensor_tensor(
                        out=db[:], in0=hd[:], scalar=0.125,
                        in1=d_ps[:], op0=Alu.mult, op1=Alu.add)
                    if dst_ap is None:
                        dst_ap = d_bnc[bb][md * 128:(md + 1) * 128, :]
                    nc.scalar.dma_start(dst_ap, db[:])

                mloc_tiles = {}
                hb_tiles = {}

                def mlp_block(b):
                    bsl = slice(b * NBLK, (b + 1) * NBLK)
                    hb_t = hbp.tile([128, KC, NBLK], bf16, name="hb",
                                    bufs=1)
                    nc.sync.dma_start(
                        hb_t[:],
                        h_sh[b][:].rearrange("(t p) s -> p t s", p=128))
                    hb_tiles[b] = hb_t
                    # LN2 row stats: sum at row 0, sumsq at row 32 of one
                    # PSUM bank (sequential accumulation groups)
                    st_ps = psD2p.tile([128, NBLK], f32, name="d_ps",
                                       tag="dst")
                    for kc in range(KC):
                        nc.tensor.matmul(
                            st_ps[0:1, :], ones_c[:, 0:1], hb_t[:, kc, :],
                            start=(kc == 0), stop=(kc == KC - 1),
                            tile_position=(0, 0))
                    for kc in range(KC):
                        sq_t = sq2p.tile([128, NBLK], bf16, name="sq2_t")
                        nc.vector.tensor_mul(sq_t[:], hb_t[:, kc, :],
                                             hb_t[:, kc, :])
                        nc.tensor.matmul(
                            st_ps[32:33, :], ones_c[:, 0:1], sq_t[:],
                            start=(kc == 0), stop=(kc == KC - 1),
                            tile_position=(0, 32))
                    # broadcast raw sums, then widened stats math
                    srow = sm2p.tile([1, NBLK], bf16, name="srow")
                    nc.scalar.activation(srow[:], st_ps[0:1, :], AF.Copy,
                                         scale=1.0 / D)
                    qrow = sm2p.tile([1, NBLK], bf16, name="qrow")
                    nc.scalar.activation(qrow[:], st_ps[32:33, :], AF.Copy,
                                         scale=1.0 / D)
                    mean_ps = psGp.tile([128, NBLK], f32, name="g_ps",
                                        tag="g")
                    nc.tensor.matmul(mean_ps[:], ones128[0:1, :], srow[:],
                                     start=True, stop=True)
                    ex2_ps = psUp.tile([128, NBLK], f32, name="u_ps",
                                       tag="u")
                    nc.tensor.matmul(ex2_ps[:], ones128[0:1, :], qrow[:],
                                     start=True, stop=True)
                    # tmp: mean^2 -> var -> sd -> 1/sd (aliased in place)
                    tmp_t = sm2p.tile([128, NBLK], f32, name="tmp")
                    nc.scalar.activation(tmp_t[:], mean_ps[:], AF.Square)
                    nc.vector.tensor_sub(tmp_t[:], ex2_ps[:], tmp_t[:])
                    nc.scalar.activation(tmp_t[:], tmp_t[:], AF.Sqrt,
                                         bias=eps_c[:])
                    nc.vector.reciprocal(tmp_t[:], tmp_t[:])
                    rstd_t = sm2p.tile([128, NBLK], bf16, name="rstd")
                    nc.vector.tensor_copy(rstd_t[:], tmp_t[:])
                    c2_t = sm2p.tile([128, NBLK], bf16, name="c2")
                    nc.vector.tensor_mul(c2_t[:], mean_ps[:], tmp_t[:])
                    h2_t = h2p.tile([128, KC, NBLK], bf16, name="h2",
                                    bufs=1)
                    for kc in range(KC):
                        nc.vector.tensor_mul(h2_t[:, kc, :], hb_t[:, kc, :],
                                             rstd_t[:])
                        nc.vector.tensor_sub(h2_t[:, kc, :], h2_t[:, kc, :],
                                             c2_t[:])
                    # g/u with streamed weights (quarters: 2 mf each)
                    m_loc = mlocp.tile([128, MF, NBLK], bf16, name="m_loc")
                    mloc_tiles[b] = m_loc
                    for mf in range(MF):
                        if mf % 2 == 0:
                            wgq = mlpwp.tile([128, KC, 256], bf16,
                                             name="wgq")
                            nc.sync.dma_start(
                                wgq[:], wg_e[:, :, mf * 128:(mf + 2) * 128])
                            wuq = mlpwp.tile([128, KC, 256], bf16,
                                             name="wuq")
                            nc.scalar.dma_start(
                                wuq[:], wu_e[:, :, mf * 128:(mf + 2) * 128])
                        wofs = (mf % 2) * 128
                        psg = psGp.tile([128, NBLK], f32, name="g_ps",
                                        tag="g")
                        for kc in range(KC):
                            nc.tensor.matmul(
                                psg[:], wgq[:, kc, wofs:wofs + 128],
                                h2_t[:, kc, :],
                                start=(kc == 0), stop=(kc == KC - 1))
                        psu = psUp.tile([128, NBLK], f32, name="u_ps",
                                        tag="u")
                        for kc in range(KC):
                            nc.tensor.matmul(
                                psu[:], wuq[:, kc, wofs:wofs + 128],
                                h2_t[:, kc, :],
                                start=(kc == 0), stop=(kc == KC - 1))
                        sg = gutp.tile([128, NBLK], bf16, name="sg")
                        nc.scalar.activation(sg[:], psg[:], AF.Silu,
                                             bias=bg_t[:, mf:mf + 1])
                        nc.vector.scalar_tensor_tensor(
                            out=m_loc[:, mf, :], in0=psu[:],
                            scalar=bu_t[:, mf:mf + 1], in1=sg[:],
                            op0=Alu.add, op1=Alu.mult)
                        # interleave previous block's down proj (2 per mf)
                        if b >= 1:
                            for md in (2 * mf, 2 * mf + 1):
                                emit_down(b - 1, md, hb_tiles[b - 1],
                                          mloc_tiles[b - 1], psD2p, "dst")

                for b in range(NB):
                    attention_block(b)
                for b in range(NB):
                    mlp_block(b)
                    if b >= 1:
                        # down of b-1 completed inside mlp_block(b)
                        nc.gpsimd.collective_compute(
                            "ReduceScatter", Alu.add, replica_groups=rg,
                            ins=[d_bnc[b - 1].opt()],
                            outs=[fin[b - 1].opt()])
                # final block's down proj, split in two RS halves: even
                # md chunks first (-> half 0), RS_3a overlaps the odd md
                # chunks, then RS_3b is the only exposed tail.
                # d3h[i] chunk c = D rows c*256 + i*128.
                for i in range(2):
                    for mdh in range(KC // 2):
                        md = 2 * mdh + i
                        pool, tag = ((psD2p, "dst") if md % 2 == 0
                                     else (psGp, "g"))
                        emit_down(
                            NB - 1, md, hb_tiles[NB - 1],
                            mloc_tiles[NB - 1], pool, tag,
                            dst_ap=d3h[i][mdh * 128:(mdh + 1) * 128, :])
                    nc.gpsimd.collective_compute(
                        "ReduceScatter", Alu.add, replica_groups=rg,
                        ins=[d3h[i].opt()], outs=[fin3[i].opt()])
                for b in range(NB - 1):
                    nc.sync.dma_start(
                        out_e[:, b * NBLK:(b + 1) * NBLK], fin[b][:])
                for i in range(2):
                    nc.sync.dma_start(
                        out_e[i * 128:(i + 1) * 128,
                              (NB - 1) * NBLK:NB * NBLK], fin3[i][:])
                if DEBUG:
                    for b in range(NB):
                        nc.sync.dma_start(
                            hdbg_e[:, b * NBLK:(b + 1) * NBLK],
                            h_sh[b][:])
            for p_ in reversed((wmatp, attwp, atttp, attrp, xhbp, mlpwp,
                                hbp, hdp, h2p, mlocp, gutp, sq2p, sm2p,
                                psSp, psAp, psDnp, psBrp, psGp, psUp,
                                psD2p)):
                p_.release()
            statkp.release()
            kqvp.release()

    return nc


# ---------------------------------------------------------------------------
# Host side
# ---------------------------------------------------------------------------

def _chunkT(a):
    """[R, D] -> [128, D//128, R] view for lhsT/rhs chunk layout.

    Result[p, kc, r] = a[r, kc*128 + p].
    """
    R, Dd = a.shape
    return np.ascontiguousarray(
        a.reshape(R, Dd // 128, 128).transpose(2, 1, 0))


def prepare_inputs(hidden_states, memory, position_ids,
                   ln1_w, ln1_b, ln2_w, ln2_b,
                   Wq, Wk, Wv, Wo, Wg, Wu, Wd, S):
    """Build the 8 per-core in_maps (numpy host prep)."""
    f32 = np.float32
    hid = np.asarray(hidden_states, f32)[0]       # [S, D]
    mem = np.asarray(memory, f32)[0]
    pos = np.asarray(position_ids)[0].astype(np.float64)

    Wq1 = np.asarray(Wq, f32) * np.asarray(ln1_w, f32)[None, :]
    Wk1 = np.asarray(Wk, f32) * np.asarray(ln1_w, f32)[None, :]
    Wv1 = np.asarray(Wv, f32) * np.asarray(ln1_w, f32)[None, :]
    bq = np.asarray(Wq, f32) @ np.asarray(ln1_b, f32)
    bk = np.asarray(Wk, f32) @ np.asarray(ln1_b, f32)
    bv = np.asarray(Wv, f32) @ np.asarray(ln1_b, f32)
    Wg2 = np.asarray(Wg, f32) * np.asarray(ln2_w, f32)[None, :]
    Wu2 = np.asarray(Wu, f32) * np.asarray(ln2_w, f32)[None, :]
    bg = np.asarray(Wg, f32) @ np.asarray(ln2_b, f32)
    bu = np.asarray(Wu, f32) @ np.asarray(ln2_b, f32)
    Wo_ = np.asarray(Wo, f32)
    Wd_ = np.asarray(Wd, f32)

    # x^T chunk layouts (shared by all cores)
    xm = _chunkT(mem).astype(BF16)                # [128, KC, S]
    xh = _chunkT(hid).astype(BF16)

    # rope tables [128, S], row pattern period 16
    inv = BASE ** (-(np.arange(8, dtype=np.float64) * 2) / RD)
    t = pos[:, None] * inv[None, :]               # [S, 8]
    cos8 = np.cos(t).T                            # [8, S]
    sin8 = np.sin(t).T
    cos16 = np.concatenate([cos8, cos8], 0)       # [16, S]
    sin16 = np.concatenate([-sin8, sin8], 0)
    cosf = np.tile(cos16, (8, 1)).astype(BF16)    # [128, S]
    sinf = np.tile(sin16, (8, 1)).astype(BF16)

    # strict-causal masks for the 4 diagonal-band offsets
    ii = np.arange(128)[:, None]
    jj = np.arange(NBLK)[None, :]
    masks = np.stack(
        [(ii + 128 * o < jj) for o in range(4)], 1).astype(BF16)

    in_maps = []
    for c in range(N_CORES):
        jsl = slice(c * J, (c + 1) * J)
        fsl = slice(c * FFL, (c + 1) * FFL)
        wq_c = Wq1[jsl]                            # [J, D]
        wk_c = Wk1[jsl]
        wv_c = Wv1[jsl]
        im = {
            "xm": xm, "xh": xh,
            "wq": _chunkT(wq_c).astype(BF16),
            "wk": _chunkT(wk_c).astype(BF16),
            "wv": _chunkT(wv_c).astype(BF16),
            "woT": _chunkT(Wo_[:, jsl]).astype(BF16),
            "wg": _chunkT(Wg2[fsl]).astype(BF16),
            "wu": _chunkT(Wu2[fsl]).astype(BF16),
            "wdT": _chunkT(Wd_[:, fsl]).astype(BF16),
            "wsq": np.ascontiguousarray(
                wq_c.sum(1).reshape(MD, 128).T).astype(f32),
            "wsk": np.ascontiguousarray(
                wk_c.sum(1).reshape(MD, 128).T).astype(f32),
            "wsvc": np.ascontiguousarray(
                wv_c.sum(1).reshape(MD, 128).T).astype(f32),
            "bq": np.ascontiguousarray(
                bq[jsl].reshape(MD, 128).T).astype(f32),
            "bk": np.ascontiguousarray(
                bk[jsl].reshape(MD, 128).T).astype(f32),
            "bvc": np.ascontiguousarray(
                bv[jsl].reshape(MD, 128).T).astype(f32),
            "wsv_row": wv_c.sum(1)[None, :].astype(f32),
            "bv_row": bv[jsl][None, :].astype(f32),
            "bg": np.ascontiguousarray(
                bg[fsl].reshape(MF, 128).T).astype(f32),
            "bu": np.ascontiguousarray(
                bu[fsl].reshape(MF, 128).T).astype(f32),
            "rope_cos": cosf, "rope_sinsg": sinf,
            "masks": masks,
        }
        in_maps.append(im)
    return in_maps


def assemble_output(results, S):
    outT = np.concatenate(
        [np.asarray(results[c]["out"]).astype(np.float32)
         for c in range(N_CORES)], 0)              # [D, S]
    return np.ascontiguousarray(outT.T).reshape(1, S, D).astype(np.float32)


_GRAPH_CACHE = {}


def get_graph(S):
    if S not in _GRAPH_CACHE:
        _GRAPH_CACHE[S] = build_graph(S)
    return _GRAPH_CACHE[S]


def kernel(hidden_states, memory, attention_mask, position_ids,
           ln1_w, ln1_b, ln2_w, ln2_b, Wq, Wk, Wv, Wo, Wg, Wu, Wd):
    from concourse.bass_utils import run_bass_kernel_spmd

    S = np.asarray(hidden_states).shape[1]
    in_maps = prepare_inputs(
        hidden_states, memory, position_ids, ln1_w, ln1_b, ln2_w, ln2_b,
        Wq, Wk, Wv, Wo, Wg, Wu, Wd, S)
    nc = get_graph(S)
    res = run_bass_kernel_spmd(nc, in_maps, core_ids=list(range(N_CORES)))
    return assemble_output(res.results, S)


# revision 74
# speedup vs baseline: 1.0810x; 1.0810x over previous
"""Trainium2 Bass kernel for nn_ArcDecoderLayer (sparse_attention).

Self-contained: takes FULL unsharded inputs, shards across 8 NeuronCores
(head-parallel attention + FF-parallel MLP), returns the FULL output.

v2 restructure vs baseline:
 - o_proj is computed locally per-core (contraction over the core's 4 heads)
   and combined with a per-block AllReduce (with hidden/8 folded in) that
   directly yields h = hidden + attn_out on every core.  This replaces the
   attn AllGather + gathered o_proj + o AllGather chain.
 - The MLP down projection is computed locally (contraction over the core's
   1024 FF dims, output = full D) and combined with a ReduceScatter per
   S-half (with h/8 folded in) that directly yields the final output slice.
   This replaces the FF-intermediate AllGather + gathered down proj.
 - Attention, o_proj+AllReduce, LN2 and the MLP are pipelined per 512-wide
   sequence block, so collectives overlap compute.
 - Softmax denominator / self-key vector work runs on full 128-partition
   tiles instead of single-row slices; LN squares run on DVE instead of
   the scalar engine; SiLU uses the fused Silu activation.
 - wg/wu weights are streamed per quarter per block to fit SBUF alongside
   the attention working set.
"""

import sys
import types

sys.path.insert(0, "/opt/trn_rl_repo")

# ---- shim antenv.axon_hooks so trace=True profiling works in this image ----
if "antenv.axon_hooks" not in sys.modules:
    _hook_mod = types.ModuleType("antenv.axon_hooks")
    _hook_state = {"hook": None}

    def _set_hook(h):
        _hook_state["hook"] = h

    def _get_hook():
        return _hook_state["hook"]

    _hook_mod.set_axon_ntff_profile_hook = _set_hook
    _hook_mod.get_axon_ntff_profile_hook = _get_hook
    sys.modules["antenv.axon_hooks"] = _hook_mod
    try:
        import antenv

        antenv.axon_hooks = _hook_mod
        from trn_agent_boot.trn_boot import _ntff_profile_via_ctypes

        _set_hook(_ntff_profile_via_ctypes("/opt/axon/libaxon_pjrt.so"))
    except Exception:
        pass

import numpy as np
import ml_dtypes

import concourse.bass as bass
import concourse.mybir as mybir
import concourse.tile as tile
from concourse import library_config
from concourse.vector_clock import ScopedClock

BF16 = ml_dtypes.bfloat16

N_CORES = 8
D = 2048
FF = 8192
H = 32
DH = 64
RD = 16
EPS = 1e-5
BASE = 10000.0

J = D // N_CORES        # 256 head-dims per core (4 heads)
FFL = FF // N_CORES     # 1024 ff dims per core
KC = D // 128           # 16 contraction chunks
NBLK = 512              # seq block width
MD = J // 128           # 2 Mtiles in the core's J slice
FC = FFL // 128         # 8 contraction chunks for the down proj
MF = FFL // 128         # 8 output Mtiles for g/u


WAIT_LIMITS = {"InstNoOp": 1, "InstDrain": 1, "InstEventSemaphore": 1}
DEFAULT_WAIT_LIMIT = 1

DEBUG = False


class PatchedTC(tile.TileContext):
    """TileContext patched for this walrus build, which rejects instructions
    carrying more than a couple of sync wait commands: excess waits are
    split onto injected same-engine nops just before the instruction."""

    _wsplit_n = 0

    def _split_excess_waits(self, ordered):
        for bb, insts in ordered.items():
            out = []
            for inst in insts:
                si = inst.sync_info
                waits = list(si.on_wait) if si and si.on_wait else []
                lim = WAIT_LIMITS.get(type(inst).__name__,
                                      DEFAULT_WAIT_LIMIT)
                if len(waits) > lim:
                    for w in waits[:-lim]:
                        nop = mybir.InstNoOp(
                            name=f"I-wsplit-{PatchedTC._wsplit_n}",
                            ins=[], outs=[], engine=inst.engine,
                            nofuse=True)
                        PatchedTC._wsplit_n += 1
                        nop.sync_info = mybir.SyncInfo(
                            on_wait=[w], on_update=[])
                        out.append(nop)
                    inst.sync_info = mybir.SyncInfo(
                        on_wait=waits[-lim:],
                        on_update=list(si.on_update or []))
                out.append(inst)
            ordered[bb] = out

    def _lower_ordered_insts(self, ordered):
        self._split_excess_waits(ordered)
        return super()._lower_ordered_insts(ordered)

    def _drain_and_barrier(self, tick_clock, wait_clock):
        nc = self.nc
        probe = nc.sync.nop(nofuse=True, hint="tail_wait_probe")
        wait_clock.add_sem_waits(
            probe.ins, ScopedClock({None: tick_clock.global_clock})
        )
        waits = list(probe.ins.sync_info.on_wait or [])
        probe.ins.sync_info.on_wait = waits[:1]
        for i in range(1, len(waits)):
            n = nc.sync.nop(nofuse=True, hint=f"tail_wait_{i}")
            n.ins.sync_info = mybir.SyncInfo(on_wait=[waits[i]], on_update=[])
        nc.sync.drain()
        nc.all_engine_barrier()
        assert self.sems is not None
        popped = nc._tile_sem_poison_stack.pop()
        assert popped is self._sem_poison
        nc.clear_and_free_semaphores(list(self.sems.allocated().values()))
        nc.all_engine_barrier()


def build_graph(S):
    """Build the SPMD 8-core graph for sequence length S (multiple of 512)."""
    dt = mybir.dt
    f32, bf16 = dt.float32, dt.bfloat16
    AF = mybir.ActivationFunctionType
    Alu = mybir.AluOpType
    NB = S // NBLK          # seq blocks
    LT = S // 128           # 128-wide l tiles
    HF = NB // 2            # ReduceScatter halves

    nc = bass.Bass()
    P = nc.declare_dram_parameter

    xm_e = P("xm", [128, KC, S], bf16, isOutput=False)
    xh_e = P("xh", [128, KC, S], bf16, isOutput=False)
    wq_e = P("wq", [128, KC, J], bf16, isOutput=False)
    wk_e = P("wk", [128, KC, J], bf16, isOutput=False)
    wv_e = P("wv", [128, KC, J], bf16, isOutput=False)
    woT_e = P("woT", [128, MD, D], bf16, isOutput=False)
    wg_e = P("wg", [128, KC, FFL], bf16, isOutput=False)
    wu_e = P("wu", [128, KC, FFL], bf16, isOutput=False)
    wdT_e = P("wdT", [128, FC, D], bf16, isOutput=False)
    # column (per-partition) weight rowsums + biases for q/k/vTh epilogues
    wsq_e = P("wsq", [128, 2], f32, isOutput=False)
    wsk_e = P("wsk", [128, 2], f32, isOutput=False)
    wsvc_e = P("wsvc", [128, 2], f32, isOutput=False)
    bq_e = P("bq", [128, 2], f32, isOutput=False)
    bk_e = P("bk", [128, 2], f32, isOutput=False)
    bvc_e = P("bvc", [128, 2], f32, isOutput=False)
    # row layouts for v_mem epilogue
    wsv_e = P("wsv_row", [1, J], f32, isOutput=False)
    bv_e = P("bv_row", [1, J], f32, isOutput=False)
    bg_e = P("bg", [128, MF], f32, isOutput=False)
    bu_e = P("bu", [128, MF], f32, isOutput=False)
    ropec_e = P("rope_cos", [128, S], bf16, isOutput=False)
    ropes_e = P("rope_sinsg", [128, S], bf16, isOutput=False)
    masks_e = P("masks", [128, 4, NBLK], bf16, isOutput=False)
    out_e = P("out", [J, S], bf16, isOutput=True)
    hdbg_e = P("hdbg", [D, S], bf16, isOutput=True) if DEBUG else None

    rg = [list(range(N_CORES))]

    with PatchedTC(nc) as tc:
        with (
            tc.tile_pool(name="const", bufs=1) as constp,
            tc.tile_pool(name="dram", bufs=1, space="DRAM") as dramp,
            tc.tile_pool(name="dsh", bufs=1, space="DRAM") as dshp,
        ):
            kqvp = tc.alloc_tile_pool(name="kqv", bufs=1)
            statkp = tc.alloc_tile_pool(name="statk", bufs=1)

            masks_t = constp.tile([128, 4, NBLK], bf16)
            nc.sync.dma_start(masks_t[:], masks_e[:])
            ones_c = constp.tile([128, 1], bf16)
            nc.vector.memset(ones_c[:], 1.0)
            ones128 = constp.tile([128, 128], bf16)
            nc.vector.memset(ones128[:], 1.0)
            eps_c = constp.tile([128, 1], f32)
            nc.vector.memset(eps_c[:], EPS)
            onesf = constp.tile([1, 128], f32)
            nc.vector.memset(onesf[:], 1.0)

            wsq_t = constp.tile([128, 2], f32)
            nc.sync.dma_start(wsq_t[:], wsq_e[:])
            wsk_t = constp.tile([128, 2], f32)
            nc.sync.dma_start(wsk_t[:], wsk_e[:])
            wsvc_t = constp.tile([128, 2], f32)
            nc.sync.dma_start(wsvc_t[:], wsvc_e[:])
            bq_t = constp.tile([128, 2], f32)
            nc.sync.dma_start(bq_t[:], bq_e[:])
            bk_t = constp.tile([128, 2], f32)
            nc.sync.dma_start(bk_t[:], bk_e[:])
            bvc_t = constp.tile([128, 2], f32)
            nc.sync.dma_start(bvc_t[:], bvc_e[:])
            bg_t = constp.tile([128, MF], f32)
            nc.sync.dma_start(bg_t[:], bg_e[:])
            bu_t = constp.tile([128, MF], f32)
            nc.sync.dma_start(bu_t[:], bu_e[:])

            def bcast_rows(dst, src_row, width, pspool, ones_row):
                """dst[0:128, :width] = src_row[0, :width] via K=1 matmuls."""
                for i in range(0, width, NBLK):
                    w = min(NBLK, width - i)
                    ps = pspool.tile([128, NBLK], f32, name="bc_ps",
                                     tag="bc_ps", bufs=1)
                    nc.tensor.matmul(ps[:, :w], ones_row[0:1, :],
                                     src_row[0:1, i:i + w],
                                     start=True, stop=True)
                    nc.vector.tensor_copy(dst[:, i:i + w], ps[:, :w])

            # persistent QKV outputs (mem-part k only; self-keys are folded
            # into the qk product and never stored)
            kT = [kqvp.tile([128, S], bf16, name=f"kT{m}") for m in range(2)]
            qT = [kqvp.tile([128, S], bf16, name=f"qT{m}") for m in range(2)]
            vTh = [kqvp.tile([128, S], bf16, name=f"vTh{m}") for m in range(2)]
            v_mem = kqvp.tile([128, LT, J], bf16)
            sf_all = kqvp.tile([128, S], f32)   # self-key raw scores, rows
            #                                     {0,32,64,96} valid

            # v_mem epilogue needs column-layout stats of the mem part
            rstd_col_mem = statkp.tile([128, LT], f32)
            c_col_mem = statkp.tile([128, LT], f32)

            # ---------- LN1 stats + QKV -----------------------------------
            def ln_stats(xpart, sqp, psp, smallp, rowp, part_name,
                         want_col):
                """Returns (rstd_col, c_col, rstd_b, c_b) for one x part.

                Stats are over the 128*KC feature dim per l column.
                Sum is accumulated column-major (N=1 matmuls); sumsq
                row-major (squares split scalar/DVE, ones as lhsT).
                """
                sum_ps = psp.tile([128, LT], f32, name="sum_ps",
                                  tag="sum_ps")
                for lt in range(LT):
                    sl = slice(lt * 128, (lt + 1) * 128)
                    for kc in range(KC):
                        nc.tensor.matmul(
                            sum_ps[:, lt:lt + 1],
                            xpart[:, kc, sl], ones_c[:],
                            start=(kc == 0), stop=(kc == KC - 1))
                sumsq_row = rowp.tile([1, S], f32, name="sumsq_row",
                                      tag="strow")
                for nb in range(NB):
                    sq_ps = psp.tile([1, NBLK], f32, name="sq_ps",
                                     tag="sq_ps")
                    for kc in range(KC):
                        sq_t = sqp.tile([128, NBLK], bf16, name="sq_t")
                        xs = xpart[:, kc, nb * NBLK:(nb + 1) * NBLK]
                        if kc % 2 == 0:
                            nc.scalar.activation(sq_t[:], xs, AF.Square)
                        else:
                            nc.vector.tensor_mul(sq_t[:], xs, xs)
                        nc.tensor.matmul(
                            sq_ps[:], ones_c[:], sq_t[:],
                            start=(kc == 0), stop=(kc == KC - 1))
                    nc.vector.tensor_copy(
                        sumsq_row[:, nb * NBLK:(nb + 1) * NBLK], sq_ps[:])
                # sumsq row -> col via DRAM
                drq = dramp.tile([S], f32, name=f"st_sq_{part_name}")
                nc.gpsimd.dma_start(
                    drq[:].rearrange("(o a) -> o a", o=1), sumsq_row[:])
                sumsq_col = smallp.tile([128, LT], f32, name="sumsq_col")
                nc.gpsimd.dma_start(
                    sumsq_col[:], drq[:].rearrange("(t p) -> p t", p=128))
                mean_c = smallp.tile([128, LT], f32, name="mean_c")
                nc.vector.tensor_scalar_mul(mean_c[:], sum_ps[:], 1.0 / D)
                ex2_c = smallp.tile([128, LT], f32, name="ex2_c")
                nc.vector.tensor_scalar_mul(ex2_c[:], sumsq_col[:], 1.0 / D)
                m2_c = smallp.tile([128, LT], f32, name="m2_c")
                nc.vector.tensor_mul(m2_c[:], mean_c[:], mean_c[:])
                var_c = smallp.tile([128, LT], f32, name="var_c")
                nc.vector.tensor_sub(var_c[:], ex2_c[:], m2_c[:])
                sd_c = smallp.tile([128, LT], f32, name="sd_c")
                nc.scalar.activation(sd_c[:], var_c[:], AF.Sqrt, bias=eps_c[:])
                rstd_c = smallp.tile([128, LT], f32, name="rstd_c")
                nc.vector.reciprocal(rstd_c[:], sd_c[:])
                c_c = smallp.tile([128, LT], f32, name="c_c")
                nc.vector.tensor_mul(c_c[:], mean_c[:], rstd_c[:])
                # col -> row roundtrip through DRAM, then partition-broadcast
                outs = []
                for nm, col in (("rstd", rstd_c), ("c", c_c)):
                    dr = dramp.tile([S], f32, name=f"st_{nm}_{part_name}")
                    nc.gpsimd.dma_start(
                        dr[:].rearrange("(t p) -> p t", p=128), col[:])
                    row = rowp.tile([1, S], f32, name=f"row_{nm}",
                                    tag="strow")
                    nc.gpsimd.dma_start(
                        row[:], dr[:].rearrange("(o a) -> o a", o=1))
                    row16 = rowp.tile([1, S], bf16, name=f"row16_{nm}",
                                      tag="strow16")
                    nc.vector.tensor_copy(row16[:], row[:])
                    bcast = rowp.tile([128, S], bf16, name=f"bc_{nm}")
                    bcast_rows(bcast, row16, S, psp, ones128)
                    outs.append(bcast)
                return rstd_c, c_c, outs[0], outs[1]

            def proj_rows(wt, dst, xpart, rstd_b, c_b, ws_t, b_t, psp,
                          cwp):
                """q/k/vTh-style projection. Raw matmul results are copied
                to dst immediately; the LN epilogue is applied in-place
                after stats are ready (cw blocks computed lazily)."""
                for m in range(2):
                    for nb in range(NB):
                        ps = psp.tile([128, NBLK], f32, name="proj_ps",
                                      tag="proj_ps", bufs=2)
                        bsl = slice(nb * NBLK, (nb + 1) * NBLK)
                        for kc in range(KC):
                            nc.tensor.matmul(
                                ps[:],
                                wt[:, kc, m * 128:(m + 1) * 128],
                                xpart[:, kc, bsl],
                                start=(kc == 0), stop=(kc == KC - 1))
                        d = dst[m][:, bsl]
                        nc.scalar.activation(d, ps[:], AF.Copy)
                for m in range(2):
                    for nb in range(NB):
                        sl = slice(nb * NBLK, (nb + 1) * NBLK)
                        cw = cwp.tile([128, NBLK], bf16, name="cw_blk",
                                      tag="cw_blk", bufs=2)
                        nc.vector.tensor_scalar(
                            out=cw[:], in0=c_b[:, sl],
                            scalar1=ws_t[:, m:m + 1],
                            scalar2=b_t[:, m:m + 1],
                            op0=Alu.mult, op1=Alu.subtract)
                        d = dst[m][:, sl]
                        nc.vector.tensor_mul(d, d, rstd_b[:, sl])
                        nc.vector.tensor_sub(d, d, cw[:])

            with (
                tc.tile_pool(name="wqkv", bufs=1) as wqkvp,
                tc.tile_pool(name="psq", bufs=1, space="PSUM") as psqp,
                tc.tile_pool(name="psst", bufs=1, space="PSUM") as psstp,
            ):
                wq_t = wqkvp.tile([128, KC, J], bf16)
                nc.sync.dma_start(wq_t[:], wq_e[:])
                wk_t = wqkvp.tile([128, KC, J], bf16)
                nc.sync.dma_start(wk_t[:], wk_e[:])
                wv_t = wqkvp.tile([128, KC, J], bf16)
                nc.sync.dma_start(wv_t[:], wv_e[:])

                ropecp = tc.alloc_tile_pool(name="ropec", bufs=1)
                cos_t = ropecp.tile([128, S], bf16)
                nc.sync.dma_start(cos_t[:], ropec_e[:])
                sin_t = ropecp.tile([128, S], bf16)
                nc.sync.dma_start(sin_t[:], ropes_e[:])
                xmp = tc.alloc_tile_pool(name="xm", bufs=1)
                xm_t = xmp.tile([128, KC, S], bf16)
                for kc in range(KC):
                    eng = nc.sync if kc % 2 == 0 else nc.scalar
                    eng.dma_start(xm_t[:, kc, :], xm_e[:, kc, :])

                def rope(dst_tiles, ropep):
                    for m in range(2):
                        t = dst_tiles[m]
                        for o in (0, 64):
                            sw = ropep.tile([128, S], bf16,
                                            name="rope_sw", bufs=1)
                            nc.gpsimd.dma_start(
                                sw[o:o + 8, :], t[o + 8:o + 16, :])
                            nc.gpsimd.dma_start(
                                sw[o + 8:o + 16, :], t[o:o + 8, :])
                            tc_ = ropep.tile([128, S], bf16,
                                             name="rope_tc", bufs=1)
                            nc.vector.tensor_mul(
                                tc_[o:o + 16, :], t[o:o + 16, :],
                                cos_t[o:o + 16, :])
                            nc.vector.tensor_mul(
                                sw[o:o + 16, :], sw[o:o + 16, :],
                                sin_t[o:o + 16, :])
                            nc.vector.tensor_add(
                                t[o:o + 16, :], tc_[o:o + 16, :],
                                sw[o:o + 16, :])

                # ----- phase M: memory part -----
                with (
                    tc.tile_pool(name="sqa", bufs=2) as sqap,
                    tc.tile_pool(name="sma", bufs=1) as smap,
                    tc.tile_pool(name="rowa", bufs=1) as rowap,
                ):
                    rs_c, c_c, rstd_bm, c_bm = ln_stats(
                        xm_t, sqap, psstp, smap, rowap, "mem", True)
                    nc.vector.tensor_copy(rstd_col_mem[:], rs_c[:])
                    nc.vector.tensor_copy(c_col_mem[:], c_c[:])
                    wsvb = smap.tile([128, J], f32)
                    wsv_row = smap.tile([1, J], f32)
                    nc.sync.dma_start(wsv_row[:], wsv_e[:])
                    bvb = smap.tile([128, J], f32)
                    bv_row = smap.tile([1, J], f32)
                    nc.sync.dma_start(bv_row[:], bv_e[:])
                    bcast_rows(wsvb, wsv_row, J, psstp, onesf)
                    bcast_rows(bvb, bv_row, J, psstp, onesf)
                    proj_rows(wk_t, kT, xm_t, rstd_bm, c_bm, wsk_t, bk_t,
                              psqp, sqap)
                    # v_mem row-major: lhsT = xm l-tile, rhs = wv
                    for lt in range(LT):
                        ps = psqp.tile([128, J], f32, name="vm_ps",
                                       tag="vm_ps", bufs=2)
                        for kc in range(KC):
                            nc.tensor.matmul(
                                ps[:],
                                xm_t[:, kc, lt * 128:(lt + 1) * 128],
                                wv_t[:, kc, :],
                                start=(kc == 0), stop=(kc == KC - 1))
                        nc.vector.tensor_copy(v_mem[:, lt, :], ps[:])
                    for lt in range(LT):
                        # cwv = c*wsv - bv in one fused op
                        cwv = sqap.tile([128, J], f32, name="cwv")
                        nc.vector.scalar_tensor_tensor(
                            out=cwv[:], in0=wsvb[:],
                            scalar=c_col_mem[:, lt:lt + 1], in1=bvb[:],
                            op0=Alu.mult, op1=Alu.subtract)
                        nc.vector.tensor_scalar_mul(
                            v_mem[:, lt, :], v_mem[:, lt, :],
                            rstd_col_mem[:, lt:lt + 1])
                        nc.vector.tensor_sub(
                            v_mem[:, lt, :], v_mem[:, lt, :], cwv[:])
                    # rope the memory keys here so it overlaps phase H
                    rope(kT, sqap)
                xmp.release()

                # ----- phase H: hidden part -----
                with (
                    tc.tile_pool(name="xh", bufs=1) as xhp,
                    tc.tile_pool(name="sqb", bufs=2) as sqbp,
                    tc.tile_pool(name="smb", bufs=1) as smbp,
                    tc.tile_pool(name="rowb", bufs=1) as rowbp,
                    tc.tile_pool(name="khp", bufs=1) as khp,
                ):
                    xh_t = xhp.tile([128, KC, S], bf16)
                    for kc in range(KC):
                        eng = nc.scalar if kc % 2 == 0 else nc.sync
                        eng.dma_start(xh_t[:, kc, :], xh_e[:, kc, :])
                    _, _, rstd_bh, c_bh = ln_stats(
                        xh_t, sqbp, psstp, smbp, rowbp, "hid", False)
                    kh = [khp.tile([128, S], bf16, name=f"kh{m}")
                          for m in range(2)]
                    # kh first, then q: the qk self product needs pre-rope
                    # q, and roping q right after its epilogue unblocks
                    # the attention S-matmuls while vTh still projects.
                    proj_rows(wk_t, kh, xh_t, rstd_bh, c_bh, wsk_t, bk_t,
                              psqp, sqbp)
                    proj_rows(wq_t, qT, xh_t, rstd_bh, c_bh, wsq_t, bq_t,
                              psqp, sqbp)

                    # self-key raw scores BEFORE RoPE (equal positions =>
                    # rotation preserves the dot product): sf_all rows
                    # {0,32,64,96} = sum over head dims of q*k_self
                    for m in range(2):
                        nc.vector.tensor_mul(kh[m][:], qT[m][:], kh[m][:])
                    rope(qT, sqbp)
                    for b in range(NB):
                        bsl = slice(b * NBLK, (b + 1) * NBLK)
                        sf_ps = psqp.tile([128, NBLK], f32, name="sf_ps",
                                          tag="proj_ps", bufs=2)
                        for m in range(2):
                            for o in (0, 64):
                                hsl = slice(o, o + 64)
                                r = 32 * (2 * m + o // 64)
                                nc.tensor.matmul(
                                    sf_ps[r:r + 1, :], ones_c[hsl, 0:1],
                                    kh[m][hsl, bsl],
                                    start=True, stop=True,
                                    tile_position=(o, r))
                        nc.scalar.activation(sf_all[:, bsl], sf_ps[:],
                                             AF.Copy)
                    proj_rows(wv_t, vTh, xh_t, rstd_bh, c_bh, wsvc_t,
                              bvc_t, psqp, sqbp)
                ropecp.release()

            # ---------- phase A: attention + local o_proj + AllReduce -----
            o_bnc = [dramp.tile([D, NBLK], bf16, name=f"o_bnc{b}")
                     for b in range(NB)]
            h_sh = [dshp.tile([D, NBLK], bf16, name=f"h_sh{b}",
                              addr_space="Shared") for b in range(NB)]
            d_bnc = [dramp.tile([D, NBLK], bf16, name=f"d_bnc{b}")
                     for b in range(NB)]
            fin = [dramp.tile([J, NBLK], bf16, name=f"fin{b}")
                   for b in range(NB)]

            wmatp = tc.alloc_tile_pool(name="wmats", bufs=1)
            woT_t = wmatp.tile([128, MD, D], bf16)
            nc.scalar.dma_start(woT_t[:], woT_e[:])
            wdT_t = wmatp.tile([128, FC, D], bf16)
            nc.scalar.dma_start(wdT_t[:], wdT_e[:])

            attwp = tc.alloc_tile_pool(name="attw", bufs=1)
            atttp = tc.alloc_tile_pool(name="attt", bufs=1)
            attrp = tc.alloc_tile_pool(name="attr", bufs=1)
            xhbp = tc.alloc_tile_pool(name="xhb", bufs=4)
            mlpwp = tc.alloc_tile_pool(name="mlpw", bufs=2)
            hbp = tc.alloc_tile_pool(name="hbp", bufs=1)
            hdp = tc.alloc_tile_pool(name="hdp", bufs=2)
            h2p = tc.alloc_tile_pool(name="h2p", bufs=1)
            mlocp = tc.alloc_tile_pool(name="mlocp", bufs=2)
            gutp = tc.alloc_tile_pool(name="gut", bufs=2)
            sq2p = tc.alloc_tile_pool(name="sq2", bufs=2)
            sm2p = tc.alloc_tile_pool(name="sm2", bufs=1)
            psSp = tc.alloc_tile_pool(name="psS", bufs=2, space="PSUM")
            psAp = tc.alloc_tile_pool(name="psA", bufs=1, space="PSUM")
            psDnp = tc.alloc_tile_pool(name="psDen", bufs=1, space="PSUM")
            psBrp = tc.alloc_tile_pool(name="psBr", bufs=1, space="PSUM")
            psGp = tc.alloc_tile_pool(name="psG", bufs=1, space="PSUM")
            psUp = tc.alloc_tile_pool(name="psU", bufs=1, space="PSUM")
            psD2p = tc.alloc_tile_pool(name="psD2", bufs=1, space="PSUM")
            if True:
                def attention_block(b):
                    bsl = slice(b * NBLK, (b + 1) * NBLK)
                    den4 = psDnp.tile([128, NBLK], f32, name="den4")
                    avs = []
                    for m in range(2):
                        ap_ps = psAp.tile([128, NBLK], f32, name="ap")
                        for o in (0, 64):
                            hsl = slice(o, o + 64)
                            r = 32 * (2 * m + o // 64)
                            rsl = slice(r, r + 1)
                            for t in range(4 * b + 4):
                                s_ps = psSp.tile([128, NBLK], f32,
                                                 name="s_ps", tag="smm")
                                nc.tensor.matmul(
                                    s_ps[:],
                                    kT[m][hsl, t * 128:(t + 1) * 128],
                                    qT[m][hsl, bsl],
                                    start=True, stop=True,
                                    tile_position=(o, 0))
                                w_t = attwp.tile([128, NBLK], bf16,
                                                 name="w_t", bufs=3)
                                nc.scalar.activation(
                                    w_t[:], s_ps[:], AF.Exp, scale=0.125)
                                if t >= 4 * b:
                                    nc.vector.tensor_mul(
                                        w_t[:], w_t[:],
                                        masks_t[:, t - 4 * b, :])
                                nc.tensor.matmul(
                                    ap_ps[hsl, :],
                                    v_mem[:, t, m * 128 + o:
                                          m * 128 + o + 64],
                                    w_t[:],
                                    start=(t == 0), stop=(t == 4 * b + 3),
                                    tile_position=(0, o))
                                nc.tensor.matmul(
                                    den4[rsl, :], ones_c[:, 0:1], w_t[:],
                                    start=(t == 0), stop=(t == 4 * b + 3),
                                    tile_position=(0, r))
                        av_sb = attwp.tile([128, NBLK], bf16,
                                           name=f"av_sb{m}", bufs=1)
                        nc.vector.tensor_copy(av_sb[:], ap_ps[:])
                        avs.append(av_sb)
                    # widened self/denominator chain (rows {0,32,64,96}
                    # meaningful, other rows harmless garbage)
                    swf = attrp.tile([128, NBLK], f32, name="swf")
                    nc.scalar.activation(swf[:], sf_all[:, bsl], AF.Exp,
                                         scale=0.125)
                    dent = attrp.tile([128, NBLK], f32, name="dent")
                    nc.vector.tensor_add(dent[:], den4[:], swf[:])
                    rcp = attrp.tile([128, NBLK], f32, name="rcp")
                    nc.vector.reciprocal(rcp[:], dent[:])
                    swb = attrp.tile([128, NBLK], bf16, name="swb")
                    nc.vector.tensor_copy(swb[:], swf[:])
                    rcpb = attrp.tile([128, NBLK], bf16, name="rcpb")
                    nc.vector.tensor_copy(rcpb[:], rcp[:])
                    cmbs = []
                    for m in range(2):
                        sb_ps = psBrp.tile([128, NBLK], f32, name="br",
                                           tag="br")
                        for o in (0, 64):
                            r = 32 * (2 * m + o // 64)
                            rsl = slice(r, r + 1)
                            nc.tensor.matmul(
                                sb_ps[o:o + 64, :], ones128[rsl, 0:64],
                                swb[rsl, :], start=True, stop=True,
                                tile_position=(r, o))
                        t0 = atttp.tile([128, NBLK], bf16, name="t0",
                                        bufs=2)
                        nc.vector.tensor_mul(t0[:], vTh[m][:, bsl],
                                             sb_ps[:])
                        rb_ps = psBrp.tile([128, NBLK], f32, name="br",
                                           tag="br")
                        for o in (0, 64):
                            r = 32 * (2 * m + o // 64)
                            rsl = slice(r, r + 1)
                            nc.tensor.matmul(
                                rb_ps[o:o + 64, :], ones128[rsl, 0:64],
                                rcpb[rsl, :], start=True, stop=True,
                                tile_position=(r, o))
                        t1 = atttp.tile([128, NBLK], bf16, name="t1",
                                        bufs=2)
                        nc.vector.tensor_add(t1[:], avs[m][:], t0[:])
                        cmb = atttp.tile([128, NBLK], bf16, name=f"cmb{m}",
                                         bufs=1)
                        nc.vector.tensor_mul(cmb[:], t1[:], rb_ps[:])
                        cmbs.append(cmb)
                    # local o_proj: out = full D, contraction over local J;
                    # fold hidden/8 so the AllReduce yields h directly
                    for md in range(KC):
                        o_ps = psSp.tile([128, NBLK], f32, name="o_ps",
                                         tag="smm")
                        nc.tensor.matmul(
                            o_ps[:], woT_t[:, 0, md * 128:(md + 1) * 128],
                            cmbs[0][:], start=True, stop=False)
                        nc.tensor.matmul(
                            o_ps[:], woT_t[:, 1, md * 128:(md + 1) * 128],
                            cmbs[1][:], start=False, stop=True)
                        xhb = xhbp.tile([128, NBLK], bf16, name="xhb")
                        nc.sync.dma_start(xhb[:], xh_e[:, md, bsl])
                        oc = atttp.tile([128, NBLK], bf16, name="oc",
                                        bufs=2)
                        nc.vector.scalar_tensor_tensor(
                            out=oc[:], in0=xhb[:], scalar=0.125,
                            in1=o_ps[:], op0=Alu.mult, op1=Alu.add)
                        nc.scalar.dma_start(
                            o_bnc[b][md * 128:(md + 1) * 128, :], oc[:])
                    nc.gpsimd.collective_compute(
                        "AllReduce", Alu.add, replica_groups=rg,
                        ins=[o_bnc[b].opt()], outs=[h_sh[b].opt()])

                def emit_down(bb, md, hb_t, m_loc, pool, tag):
                    d_ps = pool.tile([128, NBLK], f32, name="d_ps",
                                     tag=tag)
                    for fc in range(FC):
                        nc.tensor.matmul(
                            d_ps[:], wdT_t[:, fc, md * 128:(md + 1) * 128],
                            m_loc[:, fc, :],
                            start=(fc == 0), stop=(fc == FC - 1))
                    hd = hdp.tile([128, NBLK], bf16, name="hd")
                    nc.sync.dma_start(
                        hd[:],
                        h_sh[bb][md * 128:(md + 1) * 128, :])
                    db = gutp.tile([128, NBLK], bf16, name="db")
                    nc.vector.scalar_tensor_tensor(
                        out=db[:], in0=hd[:], scalar=0.125,
                        in1=d_ps[:], op0=Alu.mult, op1=Alu.add)
                    nc.scalar.dma_start(
                        d_bnc[bb][md * 128:(md + 1) * 128, :], db[:])

                mloc_tiles = {}
                hb_tiles = {}

                def mlp_block(b):
                    bsl = slice(b * NBLK, (b + 1) * NBLK)
                    hb_t = hbp.tile([128, KC, NBLK], bf16, name="hb",
                                    bufs=1)
                    nc.sync.dma_start(
                        hb_t[:],
                        h_sh[b][:].rearrange("(t p) s -> p t s", p=128))
                    hb_tiles[b] = hb_t
                    # LN2 row stats: sum at row 0, sumsq at row 32 of one
                    # PSUM bank (sequential accumulation groups)
                    st_ps = psD2p.tile([128, NBLK], f32, name="d_ps",
                                       tag="dst")
                    for kc in range(KC):
                        nc.tensor.matmul(
                            st_ps[0:1, :], ones_c[:, 0:1], hb_t[:, kc, :],
                            start=(kc == 0), stop=(kc == KC - 1),
                            tile_position=(0, 0))
                    for kc in range(KC):
                        sq_t = sq2p.tile([128, NBLK], bf16, name="sq2_t")
                        nc.vector.tensor_mul(sq_t[:], hb_t[:, kc, :],
                                             hb_t[:, kc, :])
                        nc.tensor.matmul(
                            st_ps[32:33, :], ones_c[:, 0:1], sq_t[:],
                            start=(kc == 0), stop=(kc == KC - 1),
                            tile_position=(0, 32))
                    # broadcast raw sums, then widened stats math
                    srow = sm2p.tile([1, NBLK], bf16, name="srow")
                    nc.scalar.activation(srow[:], st_ps[0:1, :], AF.Copy,
                                         scale=1.0 / D)
                    qrow = sm2p.tile([1, NBLK], bf16, name="qrow")
                    nc.scalar.activation(qrow[:], st_ps[32:33, :], AF.Copy,
                                         scale=1.0 / D)
                    mean_ps = psGp.tile([128, NBLK], f32, name="g_ps",
                                        tag="g")
                    nc.tensor.matmul(mean_ps[:], ones128[0:1, :], srow[:],
                                     start=True, stop=True)
                    ex2_ps = psUp.tile([128, NBLK], f32, name="u_ps",
                                       tag="u")
                    nc.tensor.matmul(ex2_ps[:], ones128[0:1, :], qrow[:],
                                     start=True, stop=True)
                    # tmp: mean^2 -> var -> sd -> 1/sd (aliased in place)
                    tmp_t = sm2p.tile([128, NBLK], f32, name="tmp")
                    nc.scalar.activation(tmp_t[:], mean_ps[:], AF.Square)
                    nc.vector.tensor_sub(tmp_t[:], ex2_ps[:], tmp_t[:])
                    nc.scalar.activation(tmp_t[:], tmp_t[:], AF.Sqrt,
                                         bias=eps_c[:])
                    nc.vector.reciprocal(tmp_t[:], tmp_t[:])
                    rstd_t = sm2p.tile([128, NBLK], bf16, name="rstd")
                    nc.vector.tensor_copy(rstd_t[:], tmp_t[:])
                    c2_t = sm2p.tile([128, NBLK], bf16, name="c2")
                    nc.vector.tensor_mul(c2_t[:], mean_ps[:], tmp_t[:])
                    h2_t = h2p.tile([128, KC, NBLK], bf16, name="h2",
                                    bufs=1)
                    for kc in range(KC):
                        nc.vector.tensor_mul(h2_t[:, kc, :], hb_t[:, kc, :],
                                             rstd_t[:])
                        nc.vector.tensor_sub(h2_t[:, kc, :], h2_t[:, kc, :],
                                             c2_t[:])
                    # g/u with streamed weights (quarters: 2 mf each)
                    m_loc = mlocp.tile([128, MF, NBLK], bf16, name="m_loc")
                    mloc_tiles[b] = m_loc
                    for mf in range(MF):
                        if mf % 2 == 0:
                            wgq = mlpwp.tile([128, KC, 256], bf16,
                                             name="wgq")
                            nc.sync.dma_start(
                                wgq[:], wg_e[:, :, mf * 128:(mf + 2) * 128])
                            wuq = mlpwp.tile([128, KC, 256], bf16,
                                             name="wuq")
                            nc.scalar.dma_start(
                                wuq[:], wu_e[:, :, mf * 128:(mf + 2) * 128])
                        wofs = (mf % 2) * 128
                        psg = psGp.tile([128, NBLK], f32, name="g_ps",
                                        tag="g")
                        for kc in range(KC):
                            nc.tensor.matmul(
                                psg[:], wgq[:, kc, wofs:wofs + 128],
                                h2_t[:, kc, :],
                                start=(kc == 0), stop=(kc == KC - 1))
                        psu = psUp.tile([128, NBLK], f32, name="u_ps",
                                        tag="u")
                        for kc in range(KC):
                            nc.tensor.matmul(
                                psu[:], wuq[:, kc, wofs:wofs + 128],
                                h2_t[:, kc, :],
                                start=(kc == 0), stop=(kc == KC - 1))
                        sg = gutp.tile([128, NBLK], bf16, name="sg")
                        nc.scalar.activation(sg[:], psg[:], AF.Silu,
                                             bias=bg_t[:, mf:mf + 1])
                        nc.vector.scalar_tensor_tensor(
                            out=m_loc[:, mf, :], in0=psu[:],
                            scalar=bu_t[:, mf:mf + 1], in1=sg[:],
                            op0=Alu.add, op1=Alu.mult)
                        # interleave previous block's down proj (2 per mf)
                        if b >= 1:
                            for md in (2 * mf, 2 * mf + 1):
                                emit_down(b - 1, md, hb_tiles[b - 1],
                                          mloc_tiles[b - 1], psD2p, "dst")

                for b in range(NB):
                    attention_block(b)
                for b in range(NB):
                    mlp_block(b)
                    if b >= 1:
                        # down of b-1 completed inside mlp_block(b)
                        nc.gpsimd.collective_compute(
                            "ReduceScatter", Alu.add, replica_groups=rg,
                            ins=[d_bnc[b - 1].opt()],
                            outs=[fin[b - 1].opt()])
                # final block's down proj (alternate psum pools to avoid
                # drain stalls)
                for md in range(KC):
                    pool, tag = ((psD2p, "dst") if md % 2 == 0
                                 else (psGp, "g"))
                    emit_down(NB - 1, md, hb_tiles[NB - 1],
                              mloc_tiles[NB - 1], pool, tag)
                nc.gpsimd.collective_compute(
                    "ReduceScatter", Alu.add, replica_groups=rg,
                    ins=[d_bnc[NB - 1].opt()], outs=[fin[NB - 1].opt()])
                for b in range(NB):
                    nc.sync.dma_start(
                        out_e[:, b * NBLK:(b + 1) * NBLK], fin[b][:])
                if DEBUG:
                    for b in range(NB):
                        nc.sync.dma_start(
                            hdbg_e[:, b * NBLK:(b + 1) * NBLK],
                            h_sh[b][:])
            for p_ in reversed((wmatp, attwp, atttp, attrp, xhbp, mlpwp,
                                hbp, hdp, h2p, mlocp, gutp, sq2p, sm2p,
                                psSp, psAp, psDnp, psBrp, psGp, psUp,
                                psD2p)):
                p_.release()
            statkp.release()
            kqvp.release()

    return nc


# ---------------------------------------------------------------------------
# Host side
# ---------------------------------------------------------------------------

def _chunkT(a):
    """[R, D] -> [128, D//128, R] view for lhsT/rhs chunk layout.

    Result[p, kc, r] = a[r, kc*128 + p].
    """
    R, Dd = a.shape
    return np.ascontiguousarray(
        a.reshape(R, Dd // 128, 128).transpose(2, 1, 0))


def prepare_inputs(hidden_states, memory, position_ids,
                   ln1_w, ln1_b, ln2_w, ln2_b,
                   Wq, Wk, Wv, Wo, Wg, Wu, Wd, S):
    """Build the 8 per-core in_maps (numpy host prep)."""
    f32 = np.float32
    hid = np.asarray(hidden_states, f32)[0]       # [S, D]
    mem = np.asarray(memory, f32)[0]
    pos = np.asarray(position_ids)[0].astype(np.float64)

    Wq1 = np.asarray(Wq, f32) * np.asarray(ln1_w, f32)[None, :]
    Wk1 = np.asarray(Wk, f32) * np.asarray(ln1_w, f32)[None, :]
    Wv1 = np.asarray(Wv, f32) * np.asarray(ln1_w, f32)[None, :]
    bq = np.asarray(Wq, f32) @ np.asarray(ln1_b, f32)
    bk = np.asarray(Wk, f32) @ np.asarray(ln1_b, f32)
    bv = np.asarray(Wv, f32) @ np.asarray(ln1_b, f32)
    Wg2 = np.asarray(Wg, f32) * np.asarray(ln2_w, f32)[None, :]
    Wu2 = np.asarray(Wu, f32) * np.asarray(ln2_w, f32)[None, :]
    bg = np.asarray(Wg, f32) @ np.asarray(ln2_b, f32)
    bu = np.asarray(Wu, f32) @ np.asarray(ln2_b, f32)
    Wo_ = np.asarray(Wo, f32)
    Wd_ = np.asarray(Wd, f32)

    # x^T chunk layouts (shared by all cores)
    xm = _chunkT(mem).astype(BF16)                # [128, KC, S]
    xh = _chunkT(hid).astype(BF16)

    # rope tables [128, S], row pattern period 16
    inv = BASE ** (-(np.arange(8, dtype=np.float64) * 2) / RD)
    t = pos[:, None] * inv[None, :]               # [S, 8]
    cos8 = np.cos(t).T                            # [8, S]
    sin8 = np.sin(t).T
    cos16 = np.concatenate([cos8, cos8], 0)       # [16, S]
    sin16 = np.concatenate([-sin8, sin8], 0)
    cosf = np.tile(cos16, (8, 1)).astype(BF16)    # [128, S]
    sinf = np.tile(sin16, (8, 1)).astype(BF16)

    # strict-causal masks for the 4 diagonal-band offsets
    ii = np.arange(128)[:, None]
    jj = np.arange(NBLK)[None, :]
    masks = np.stack(
        [(ii + 128 * o < jj) for o in range(4)], 1).astype(BF16)

    in_maps = []
    for c in range(N_CORES):
        jsl = slice(c * J, (c + 1) * J)
        fsl = slice(c * FFL, (c + 1) * FFL)
        wq_c = Wq1[jsl]                            # [J, D]
        wk_c = Wk1[jsl]
        wv_c = Wv1[jsl]
        im = {
            "xm": xm, "xh": xh,
            "wq": _chunkT(wq_c).astype(BF16),
            "wk": _chunkT(wk_c).astype(BF16),
            "wv": _chunkT(wv_c).astype(BF16),
            "woT": _chunkT(Wo_[:, jsl]).astype(BF16),
            "wg": _chunkT(Wg2[fsl]).astype(BF16),
            "wu": _chunkT(Wu2[fsl]).astype(BF16),
            "wdT": _chunkT(Wd_[:, fsl]).astype(BF16),
            "wsq": np.ascontiguousarray(
                wq_c.sum(1).reshape(MD, 128).T).astype(f32),
            "wsk": np.ascontiguousarray(
                wk_c.sum(1).reshape(MD, 128).T).astype(f32),
            "wsvc": np.ascontiguousarray(
                wv_c.sum(1).reshape(MD, 128).T).astype(f32),
            "bq": np.ascontiguousarray(
                bq[jsl].reshape(MD, 128).T).astype(f32),
            "bk": np.ascontiguousarray(
                bk[jsl].reshape(MD, 128).T).astype(f32),
            "bvc": np.ascontiguousarray(
                bv[jsl].reshape(MD, 128).T).astype(f32),
            "wsv_row": wv_c.sum(1)[None, :].astype(f32),
            "bv_row": bv[jsl][None, :].astype(f32),
            "bg": np.ascontiguousarray(
                bg[fsl].reshape(MF, 128).T).astype(f32),
            "bu": np.ascontiguousarray(
                bu[fsl].reshape(MF, 128).T).astype(f32),
            "rope_cos": cosf, "rope_sinsg": sinf,
            "masks": masks,
        }
        in_maps.append(im)
    return in_maps


def assemble_output(results, S):
    outT = np.concatenate(
        [np.asarray(results[c]["out"]).astype(np.float32)
         for c in range(N_CORES)], 0)              # [D, S]
    return np.ascontiguousarray(outT.T).reshape(1, S, D).astype(np.float32)


_GRAPH_CACHE = {}


def get_graph(S):
    if S not in _GRAPH_CACHE:
        _GRAPH_CACHE[S] = build_graph(S)
    return _GRAPH_CACHE[S]


def kernel(hidden_states, memory, attention_mask, position_ids,
           ln1_w, ln1_b, ln2_w, ln2_b, Wq, Wk, Wv, Wo, Wg, Wu, Wd):
    from concourse.bass_utils import run_bass_kernel_spmd

    S = np.asarray(hidden_states).shape[1]
    in_maps = prepare_inputs(
        hidden_states, memory, position_ids, ln1_w, ln1_b, ln2_w, ln2_b,
        Wq, Wk, Wv, Wo, Wg, Wu, Wd, S)
    nc = get_graph(S)
    res = run_bass_kernel_spmd(nc, in_maps, core_ids=list(range(N_CORES)))
    return assemble_output(res.results, S)


# revision 78
# speedup vs baseline: 1.1079x; 1.0248x over previous
"""Trainium2 Bass kernel for nn_ArcDecoderLayer (sparse_attention).

Self-contained: takes FULL unsharded inputs, shards across 8 NeuronCores
(head-parallel attention + FF-parallel MLP), returns the FULL output.

v2 restructure vs baseline:
 - o_proj is computed locally per-core (contraction over the core's 4 heads)
   and combined with a per-block AllReduce (with hidden/8 folded in) that
   directly yields h = hidden + attn_out on every core.  This replaces the
   attn AllGather + gathered o_proj + o AllGather chain.
 - The MLP down projection is computed locally (contraction over the core's
   1024 FF dims, output = full D) and combined with a ReduceScatter per
   S-half (with h/8 folded in) that directly yields the final output slice.
   This replaces the FF-intermediate AllGather + gathered down proj.
 - Attention, o_proj+AllReduce, LN2 and the MLP are pipelined per 512-wide
   sequence block, so collectives overlap compute.
 - Softmax denominator / self-key vector work runs on full 128-partition
   tiles instead of single-row slices; LN squares run on DVE instead of
   the scalar engine; SiLU uses the fused Silu activation.
 - wg/wu weights are streamed per quarter per block to fit SBUF alongside
   the attention working set.
"""

import sys
import types

sys.path.insert(0, "/opt/trn_rl_repo")

# ---- shim antenv.axon_hooks so trace=True profiling works in this image ----
if "antenv.axon_hooks" not in sys.modules:
    _hook_mod = types.ModuleType("antenv.axon_hooks")
    _hook_state = {"hook": None}

    def _set_hook(h):
        _hook_state["hook"] = h

    def _get_hook():
        return _hook_state["hook"]

    _hook_mod.set_axon_ntff_profile_hook = _set_hook
    _hook_mod.get_axon_ntff_profile_hook = _get_hook
    sys.modules["antenv.axon_hooks"] = _hook_mod
    try:
        import antenv

        antenv.axon_hooks = _hook_mod
        from trn_agent_boot.trn_boot import _ntff_profile_via_ctypes

        _set_hook(_ntff_profile_via_ctypes("/opt/axon/libaxon_pjrt.so"))
    except Exception:
        pass

import numpy as np
import ml_dtypes

import concourse.bass as bass
import concourse.mybir as mybir
import concourse.tile as tile
from concourse import library_config
from concourse.vector_clock import ScopedClock

BF16 = ml_dtypes.bfloat16

N_CORES = 8
D = 2048
FF = 8192
H = 32
DH = 64
RD = 16
EPS = 1e-5
BASE = 10000.0

J = D // N_CORES        # 256 head-dims per core (4 heads)
FFL = FF // N_CORES     # 1024 ff dims per core
KC = D // 128           # 16 contraction chunks
NBLK = 512              # seq block width
MD = J // 128           # 2 Mtiles in the core's J slice
FC = FFL // 128         # 8 contraction chunks for the down proj
MF = FFL // 128         # 8 output Mtiles for g/u


WAIT_LIMITS = {"InstNoOp": 1, "InstDrain": 1, "InstEventSemaphore": 1}
DEFAULT_WAIT_LIMIT = 1

DEBUG = False


class PatchedTC(tile.TileContext):
    """TileContext patched for this walrus build, which rejects instructions
    carrying more than a couple of sync wait commands: excess waits are
    split onto injected same-engine nops just before the instruction."""

    _wsplit_n = 0

    def _split_excess_waits(self, ordered):
        for bb, insts in ordered.items():
            out = []
            for inst in insts:
                si = inst.sync_info
                waits = list(si.on_wait) if si and si.on_wait else []
                lim = WAIT_LIMITS.get(type(inst).__name__,
                                      DEFAULT_WAIT_LIMIT)
                if len(waits) > lim:
                    for w in waits[:-lim]:
                        nop = mybir.InstNoOp(
                            name=f"I-wsplit-{PatchedTC._wsplit_n}",
                            ins=[], outs=[], engine=inst.engine,
                            nofuse=True)
                        PatchedTC._wsplit_n += 1
                        nop.sync_info = mybir.SyncInfo(
                            on_wait=[w], on_update=[])
                        out.append(nop)
                    inst.sync_info = mybir.SyncInfo(
                        on_wait=waits[-lim:],
                        on_update=list(si.on_update or []))
                out.append(inst)
            ordered[bb] = out

    def _lower_ordered_insts(self, ordered):
        self._split_excess_waits(ordered)
        return super()._lower_ordered_insts(ordered)

    def _drain_and_barrier(self, tick_clock, wait_clock):
        nc = self.nc
        probe = nc.sync.nop(nofuse=True, hint="tail_wait_probe")
        wait_clock.add_sem_waits(
            probe.ins, ScopedClock({None: tick_clock.global_clock})
        )
        waits = list(probe.ins.sync_info.on_wait or [])
        probe.ins.sync_info.on_wait = waits[:1]
        for i in range(1, len(waits)):
            n = nc.sync.nop(nofuse=True, hint=f"tail_wait_{i}")
            n.ins.sync_info = mybir.SyncInfo(on_wait=[waits[i]], on_update=[])
        nc.sync.drain()
        nc.all_engine_barrier()
        assert self.sems is not None
        popped = nc._tile_sem_poison_stack.pop()
        assert popped is self._sem_poison
        nc.clear_and_free_semaphores(list(self.sems.allocated().values()))
        nc.all_engine_barrier()


def build_graph(S):
    """Build the SPMD 8-core graph for sequence length S (multiple of 512)."""
    dt = mybir.dt
    f32, bf16 = dt.float32, dt.bfloat16
    AF = mybir.ActivationFunctionType
    Alu = mybir.AluOpType
    NB = S // NBLK          # seq blocks
    LT = S // 128           # 128-wide l tiles
    HF = NB // 2            # ReduceScatter halves

    nc = bass.Bass()
    P = nc.declare_dram_parameter

    xm_e = P("xm", [128, KC, S], bf16, isOutput=False)
    xh_e = P("xh", [128, KC, S], bf16, isOutput=False)
    wq_e = P("wq", [128, KC, J], bf16, isOutput=False)
    wk_e = P("wk", [128, KC, J], bf16, isOutput=False)
    wv_e = P("wv", [128, KC, J], bf16, isOutput=False)
    woT_e = P("woT", [128, MD, D], bf16, isOutput=False)
    wg_e = P("wg", [128, KC, FFL], bf16, isOutput=False)
    wu_e = P("wu", [128, KC, FFL], bf16, isOutput=False)
    wdT_e = P("wdT", [128, FC, D], bf16, isOutput=False)
    # column (per-partition) weight rowsums + biases for q/k/vTh epilogues
    wsq_e = P("wsq", [128, 2], f32, isOutput=False)
    wsk_e = P("wsk", [128, 2], f32, isOutput=False)
    wsvc_e = P("wsvc", [128, 2], f32, isOutput=False)
    bq_e = P("bq", [128, 2], f32, isOutput=False)
    bk_e = P("bk", [128, 2], f32, isOutput=False)
    bvc_e = P("bvc", [128, 2], f32, isOutput=False)
    # row layouts for v_mem epilogue
    wsv_e = P("wsv_row", [1, J], f32, isOutput=False)
    bv_e = P("bv_row", [1, J], f32, isOutput=False)
    bg_e = P("bg", [128, MF], f32, isOutput=False)
    bu_e = P("bu", [128, MF], f32, isOutput=False)
    ropec_e = P("rope_cos", [128, S], bf16, isOutput=False)
    ropes_e = P("rope_sinsg", [128, S], bf16, isOutput=False)
    masks_e = P("masks", [128, 4, NBLK], bf16, isOutput=False)
    out_e = P("out", [J, S], bf16, isOutput=True)
    hdbg_e = P("hdbg", [D, S], bf16, isOutput=True) if DEBUG else None

    rg = [list(range(N_CORES))]

    with PatchedTC(nc) as tc:
        with (
            tc.tile_pool(name="const", bufs=1) as constp,
            tc.tile_pool(name="dram", bufs=1, space="DRAM") as dramp,
            tc.tile_pool(name="dsh", bufs=1, space="DRAM") as dshp,
        ):
            kqvp = tc.alloc_tile_pool(name="kqv", bufs=1)
            statkp = tc.alloc_tile_pool(name="statk", bufs=1)

            masks_t = constp.tile([128, 4, NBLK], bf16)
            nc.sync.dma_start(masks_t[:], masks_e[:])
            ones_c = constp.tile([128, 1], bf16)
            nc.vector.memset(ones_c[:], 1.0)
            ones128 = constp.tile([128, 128], bf16)
            nc.vector.memset(ones128[:], 1.0)
            eps_c = constp.tile([128, 1], f32)
            nc.vector.memset(eps_c[:], EPS)
            onesf = constp.tile([1, 128], f32)
            nc.vector.memset(onesf[:], 1.0)

            wsq_t = constp.tile([128, 2], f32)
            nc.sync.dma_start(wsq_t[:], wsq_e[:])
            wsk_t = constp.tile([128, 2], f32)
            nc.sync.dma_start(wsk_t[:], wsk_e[:])
            wsvc_t = constp.tile([128, 2], f32)
            nc.sync.dma_start(wsvc_t[:], wsvc_e[:])
            bq_t = constp.tile([128, 2], f32)
            nc.sync.dma_start(bq_t[:], bq_e[:])
            bk_t = constp.tile([128, 2], f32)
            nc.sync.dma_start(bk_t[:], bk_e[:])
            bvc_t = constp.tile([128, 2], f32)
            nc.sync.dma_start(bvc_t[:], bvc_e[:])
            bg_t = constp.tile([128, MF], f32)
            nc.sync.dma_start(bg_t[:], bg_e[:])
            bu_t = constp.tile([128, MF], f32)
            nc.sync.dma_start(bu_t[:], bu_e[:])

            def bcast_rows(dst, src_row, width, pspool, ones_row):
                """dst[0:128, :width] = src_row[0, :width] via K=1 matmuls."""
                for i in range(0, width, NBLK):
                    w = min(NBLK, width - i)
                    ps = pspool.tile([128, NBLK], f32, name="bc_ps",
                                     tag="bc_ps", bufs=1)
                    nc.tensor.matmul(ps[:, :w], ones_row[0:1, :],
                                     src_row[0:1, i:i + w],
                                     start=True, stop=True)
                    nc.vector.tensor_copy(dst[:, i:i + w], ps[:, :w])

            # persistent QKV outputs (mem-part k only; self-keys are folded
            # into the qk product and never stored)
            kT = [kqvp.tile([128, S], bf16, name=f"kT{m}") for m in range(2)]
            qT = [kqvp.tile([128, S], bf16, name=f"qT{m}") for m in range(2)]
            vTh = [kqvp.tile([128, S], bf16, name=f"vTh{m}") for m in range(2)]
            v_mem = kqvp.tile([128, LT, J], bf16)
            sf_all = kqvp.tile([128, S], f32)   # self-key raw scores, rows
            #                                     {0,32,64,96} valid

            # v_mem epilogue needs column-layout stats of the mem part
            rstd_col_mem = statkp.tile([128, LT], f32)
            c_col_mem = statkp.tile([128, LT], f32)

            # ---------- LN1 stats + QKV -----------------------------------
            def ln_stats(xpart, sqp, psp, smallp, rowp, part_name,
                         want_col):
                """Returns (rstd_col, c_col, rstd_b, c_b) for one x part.

                Stats are over the 128*KC feature dim per l column.
                Sum is accumulated column-major (N=1 matmuls); sumsq
                row-major (squares split scalar/DVE, ones as lhsT).
                """
                sum_ps = psp.tile([128, LT], f32, name="sum_ps",
                                  tag="sum_ps")
                for lt in range(LT):
                    sl = slice(lt * 128, (lt + 1) * 128)
                    for kc in range(KC):
                        nc.tensor.matmul(
                            sum_ps[:, lt:lt + 1],
                            xpart[:, kc, sl], ones_c[:],
                            start=(kc == 0), stop=(kc == KC - 1))
                sumsq_row = rowp.tile([1, S], f32, name="sumsq_row",
                                      tag="strow")
                for nb in range(NB):
                    sq_ps = psp.tile([1, NBLK], f32, name="sq_ps",
                                     tag="sq_ps")
                    for kc in range(KC):
                        sq_t = sqp.tile([128, NBLK], bf16, name="sq_t")
                        xs = xpart[:, kc, nb * NBLK:(nb + 1) * NBLK]
                        if kc % 2 == 0:
                            nc.scalar.activation(sq_t[:], xs, AF.Square)
                        else:
                            nc.vector.tensor_mul(sq_t[:], xs, xs)
                        nc.tensor.matmul(
                            sq_ps[:], ones_c[:], sq_t[:],
                            start=(kc == 0), stop=(kc == KC - 1))
                    nc.vector.tensor_copy(
                        sumsq_row[:, nb * NBLK:(nb + 1) * NBLK], sq_ps[:])
                # sumsq row -> col via DRAM
                drq = dramp.tile([S], f32, name=f"st_sq_{part_name}")
                nc.gpsimd.dma_start(
                    drq[:].rearrange("(o a) -> o a", o=1), sumsq_row[:])
                sumsq_col = smallp.tile([128, LT], f32, name="sumsq_col")
                nc.gpsimd.dma_start(
                    sumsq_col[:], drq[:].rearrange("(t p) -> p t", p=128))
                return sum_ps, sumsq_col

            def ln_finish(sum_ps, sumsq_col, smallp, rowp, psp,
                          part_name):
                """Roundtrip-dependent stats math + broadcasts. Emitted
                AFTER the raw projection matmuls so the DMA-roundtrip
                wait never blocks PSUM-drain work queued behind it."""
                mean_c = smallp.tile([128, LT], f32, name="mean_c")
                nc.vector.tensor_scalar_mul(mean_c[:], sum_ps[:], 1.0 / D)
                ex2_c = smallp.tile([128, LT], f32, name="ex2_c")
                nc.vector.tensor_scalar_mul(ex2_c[:], sumsq_col[:], 1.0 / D)
                m2_c = smallp.tile([128, LT], f32, name="m2_c")
                nc.vector.tensor_mul(m2_c[:], mean_c[:], mean_c[:])
                var_c = smallp.tile([128, LT], f32, name="var_c")
                nc.vector.tensor_sub(var_c[:], ex2_c[:], m2_c[:])
                sd_c = smallp.tile([128, LT], f32, name="sd_c")
                nc.scalar.activation(sd_c[:], var_c[:], AF.Sqrt, bias=eps_c[:])
                rstd_c = smallp.tile([128, LT], f32, name="rstd_c")
                nc.vector.reciprocal(rstd_c[:], sd_c[:])
                c_c = smallp.tile([128, LT], f32, name="c_c")
                nc.vector.tensor_mul(c_c[:], mean_c[:], rstd_c[:])
                # col -> row roundtrip through DRAM, then partition-broadcast
                outs = []
                for nm, col in (("rstd", rstd_c), ("c", c_c)):
                    dr = dramp.tile([S], f32, name=f"st_{nm}_{part_name}")
                    nc.gpsimd.dma_start(
                        dr[:].rearrange("(t p) -> p t", p=128), col[:])
                    row = rowp.tile([1, S], f32, name=f"row_{nm}",
                                    tag="strow")
                    nc.gpsimd.dma_start(
                        row[:], dr[:].rearrange("(o a) -> o a", o=1))
                    row16 = rowp.tile([1, S], bf16, name=f"row16_{nm}",
                                      tag="strow16")
                    nc.vector.tensor_copy(row16[:], row[:])
                    bcast = rowp.tile([128, S], bf16, name=f"bc_{nm}")
                    bcast_rows(bcast, row16, S, psp, ones128)
                    outs.append(bcast)
                return rstd_c, c_c, outs[0], outs[1]

            def proj_raw(wt, dst, xpart, psp):
                """Raw projection matmuls; PSUM drained to dst via the
                scalar engine (no stats dependency)."""
                for m in range(2):
                    for nb in range(NB):
                        ps = psp.tile([128, NBLK], f32, name="proj_ps",
                                      tag="proj_ps", bufs=2)
                        bsl = slice(nb * NBLK, (nb + 1) * NBLK)
                        for kc in range(KC):
                            nc.tensor.matmul(
                                ps[:],
                                wt[:, kc, m * 128:(m + 1) * 128],
                                xpart[:, kc, bsl],
                                start=(kc == 0), stop=(kc == KC - 1))
                        d = dst[m][:, bsl]
                        nc.scalar.activation(d, ps[:], AF.Copy)

            def proj_epi(dst, rstd_b, c_b, ws_t, b_t, cwp):
                """In-place LN epilogue once stats are ready."""
                for m in range(2):
                    for nb in range(NB):
                        sl = slice(nb * NBLK, (nb + 1) * NBLK)
                        cw = cwp.tile([128, NBLK], bf16, name="cw_blk",
                                      tag="cw_blk", bufs=2)
                        nc.vector.tensor_scalar(
                            out=cw[:], in0=c_b[:, sl],
                            scalar1=ws_t[:, m:m + 1],
                            scalar2=b_t[:, m:m + 1],
                            op0=Alu.mult, op1=Alu.subtract)
                        d = dst[m][:, sl]
                        nc.vector.tensor_mul(d, d, rstd_b[:, sl])
                        nc.vector.tensor_sub(d, d, cw[:])

            with (
                tc.tile_pool(name="wqkv", bufs=1) as wqkvp,
                tc.tile_pool(name="psq", bufs=1, space="PSUM") as psqp,
                tc.tile_pool(name="psst", bufs=1, space="PSUM") as psstp,
            ):
                wq_t = wqkvp.tile([128, KC, J], bf16)
                nc.sync.dma_start(wq_t[:], wq_e[:])
                wk_t = wqkvp.tile([128, KC, J], bf16)
                nc.sync.dma_start(wk_t[:], wk_e[:])
                wv_t = wqkvp.tile([128, KC, J], bf16)
                nc.sync.dma_start(wv_t[:], wv_e[:])

                ropecp = tc.alloc_tile_pool(name="ropec", bufs=1)
                cos_t = ropecp.tile([128, S], bf16)
                nc.sync.dma_start(cos_t[:], ropec_e[:])
                sin_t = ropecp.tile([128, S], bf16)
                nc.sync.dma_start(sin_t[:], ropes_e[:])
                xmp = tc.alloc_tile_pool(name="xm", bufs=1)
                xm_t = xmp.tile([128, KC, S], bf16)
                for kc in range(KC):
                    eng = nc.sync if kc % 2 == 0 else nc.scalar
                    eng.dma_start(xm_t[:, kc, :], xm_e[:, kc, :])

                def rope(dst_tiles, ropep):
                    for m in range(2):
                        t = dst_tiles[m]
                        for o in (0, 64):
                            sw = ropep.tile([128, S], bf16,
                                            name="rope_sw", bufs=1)
                            nc.gpsimd.dma_start(
                                sw[o:o + 8, :], t[o + 8:o + 16, :])
                            nc.gpsimd.dma_start(
                                sw[o + 8:o + 16, :], t[o:o + 8, :])
                            tc_ = ropep.tile([128, S], bf16,
                                             name="rope_tc", bufs=1)
                            nc.vector.tensor_mul(
                                tc_[o:o + 16, :], t[o:o + 16, :],
                                cos_t[o:o + 16, :])
                            nc.vector.tensor_mul(
                                sw[o:o + 16, :], sw[o:o + 16, :],
                                sin_t[o:o + 16, :])
                            nc.vector.tensor_add(
                                t[o:o + 16, :], tc_[o:o + 16, :],
                                sw[o:o + 16, :])

                # ----- phase M: memory part -----
                with (
                    tc.tile_pool(name="sqa", bufs=2) as sqap,
                    tc.tile_pool(name="sma", bufs=1) as smap,
                    tc.tile_pool(name="rowa", bufs=1) as rowap,
                ):
                    sum_m, sqcol_m = ln_stats(
                        xm_t, sqap, psstp, smap, rowap, "mem", True)
                    wsvb = smap.tile([128, J], f32)
                    wsv_row = smap.tile([1, J], f32)
                    nc.sync.dma_start(wsv_row[:], wsv_e[:])
                    bvb = smap.tile([128, J], f32)
                    bv_row = smap.tile([1, J], f32)
                    nc.sync.dma_start(bv_row[:], bv_e[:])
                    bcast_rows(wsvb, wsv_row, J, psstp, onesf)
                    bcast_rows(bvb, bv_row, J, psstp, onesf)
                    # all raw matmuls first (no stats dependency) ...
                    proj_raw(wk_t, kT, xm_t, psqp)
                    # v_mem row-major: lhsT = xm l-tile, rhs = wv
                    for lt in range(LT):
                        ps = psqp.tile([128, J], f32, name="vm_ps",
                                       tag="vm_ps", bufs=2)
                        for kc in range(KC):
                            nc.tensor.matmul(
                                ps[:],
                                xm_t[:, kc, lt * 128:(lt + 1) * 128],
                                wv_t[:, kc, :],
                                start=(kc == 0), stop=(kc == KC - 1))
                        nc.scalar.activation(v_mem[:, lt, :], ps[:],
                                             AF.Copy)
                    # ... then the roundtrip-dependent math + epilogues
                    rs_c, c_c, rstd_bm, c_bm = ln_finish(
                        sum_m, sqcol_m, smap, rowap, psstp, "mem")
                    nc.vector.tensor_copy(rstd_col_mem[:], rs_c[:])
                    nc.vector.tensor_copy(c_col_mem[:], c_c[:])
                    proj_epi(kT, rstd_bm, c_bm, wsk_t, bk_t, sqap)
                    for lt in range(LT):
                        # cwv = c*wsv - bv in one fused op
                        cwv = sqap.tile([128, J], f32, name="cwv")
                        nc.vector.scalar_tensor_tensor(
                            out=cwv[:], in0=wsvb[:],
                            scalar=c_col_mem[:, lt:lt + 1], in1=bvb[:],
                            op0=Alu.mult, op1=Alu.subtract)
                        nc.vector.tensor_scalar_mul(
                            v_mem[:, lt, :], v_mem[:, lt, :],
                            rstd_col_mem[:, lt:lt + 1])
                        nc.vector.tensor_sub(
                            v_mem[:, lt, :], v_mem[:, lt, :], cwv[:])
                    # rope the memory keys here so it overlaps phase H
                    rope(kT, sqap)
                xmp.release()

                # ----- phase H: hidden part -----
                with (
                    tc.tile_pool(name="xh", bufs=1) as xhp,
                    tc.tile_pool(name="sqb", bufs=2) as sqbp,
                    tc.tile_pool(name="smb", bufs=1) as smbp,
                    tc.tile_pool(name="rowb", bufs=1) as rowbp,
                    tc.tile_pool(name="khp", bufs=1) as khp,
                ):
                    xh_t = xhp.tile([128, KC, S], bf16)
                    for kc in range(KC):
                        eng = nc.scalar if kc % 2 == 0 else nc.sync
                        eng.dma_start(xh_t[:, kc, :], xh_e[:, kc, :])
                    sum_h, sqcol_h = ln_stats(
                        xh_t, sqbp, psstp, smbp, rowbp, "hid", False)
                    kh = [khp.tile([128, S], bf16, name=f"kh{m}")
                          for m in range(2)]
                    # all raw projections first; the stats-roundtrip wait
                    # then overlaps them instead of blocking PSUM drains
                    proj_raw(wk_t, kh, xh_t, psqp)
                    proj_raw(wq_t, qT, xh_t, psqp)
                    proj_raw(wv_t, vTh, xh_t, psqp)
                    _, _, rstd_bh, c_bh = ln_finish(
                        sum_h, sqcol_h, smbp, rowbp, psstp, "hid")
                    proj_epi(kh, rstd_bh, c_bh, wsk_t, bk_t, sqbp)
                    proj_epi(qT, rstd_bh, c_bh, wsq_t, bq_t, sqbp)

                    # self-key raw scores BEFORE RoPE (equal positions =>
                    # rotation preserves the dot product): sf_all rows
                    # {0,32,64,96} = sum over head dims of q*k_self
                    for m in range(2):
                        nc.vector.tensor_mul(kh[m][:], qT[m][:], kh[m][:])
                    rope(qT, sqbp)
                    for b in range(NB):
                        bsl = slice(b * NBLK, (b + 1) * NBLK)
                        sf_ps = psqp.tile([128, NBLK], f32, name="sf_ps",
                                          tag="proj_ps", bufs=2)
                        for m in range(2):
                            for o in (0, 64):
                                hsl = slice(o, o + 64)
                                r = 32 * (2 * m + o // 64)
                                nc.tensor.matmul(
                                    sf_ps[r:r + 1, :], ones_c[hsl, 0:1],
                                    kh[m][hsl, bsl],
                                    start=True, stop=True,
                                    tile_position=(o, r))
                        nc.scalar.activation(sf_all[:, bsl], sf_ps[:],
                                             AF.Copy)
                    proj_epi(vTh, rstd_bh, c_bh, wsvc_t, bvc_t, sqbp)
                ropecp.release()

            # ---------- phase A: attention + local o_proj + AllReduce -----
            o_bnc = [dramp.tile([D, NBLK], bf16, name=f"o_bnc{b}")
                     for b in range(NB)]
            h_sh = [dshp.tile([D, NBLK], bf16, name=f"h_sh{b}",
                              addr_space="Shared") for b in range(NB)]
            d_bnc = [dramp.tile([D, NBLK], bf16, name=f"d_bnc{b}")
                     for b in range(NB)]
            fin = [dramp.tile([J, NBLK], bf16, name=f"fin{b}")
                   for b in range(NB)]

            wmatp = tc.alloc_tile_pool(name="wmats", bufs=1)
            woT_t = wmatp.tile([128, MD, D], bf16)
            nc.scalar.dma_start(woT_t[:], woT_e[:])
            wdT_t = wmatp.tile([128, FC, D], bf16)
            nc.scalar.dma_start(wdT_t[:], wdT_e[:])

            attwp = tc.alloc_tile_pool(name="attw", bufs=1)
            atttp = tc.alloc_tile_pool(name="attt", bufs=1)
            attrp = tc.alloc_tile_pool(name="attr", bufs=1)
            xhbp = tc.alloc_tile_pool(name="xhb", bufs=4)
            mlpwp = tc.alloc_tile_pool(name="mlpw", bufs=2)
            hbp = tc.alloc_tile_pool(name="hbp", bufs=1)
            hdp = tc.alloc_tile_pool(name="hdp", bufs=2)
            h2p = tc.alloc_tile_pool(name="h2p", bufs=1)
            mlocp = tc.alloc_tile_pool(name="mlocp", bufs=2)
            gutp = tc.alloc_tile_pool(name="gut", bufs=2)
            sq2p = tc.alloc_tile_pool(name="sq2", bufs=2)
            sm2p = tc.alloc_tile_pool(name="sm2", bufs=1)
            psSp = tc.alloc_tile_pool(name="psS", bufs=2, space="PSUM")
            psAp = tc.alloc_tile_pool(name="psA", bufs=1, space="PSUM")
            psDnp = tc.alloc_tile_pool(name="psDen", bufs=1, space="PSUM")
            psBrp = tc.alloc_tile_pool(name="psBr", bufs=1, space="PSUM")
            psGp = tc.alloc_tile_pool(name="psG", bufs=1, space="PSUM")
            psUp = tc.alloc_tile_pool(name="psU", bufs=1, space="PSUM")
            psD2p = tc.alloc_tile_pool(name="psD2", bufs=1, space="PSUM")
            if True:
                def attention_block(b):
                    bsl = slice(b * NBLK, (b + 1) * NBLK)
                    den4 = psDnp.tile([128, NBLK], f32, name="den4")
                    avs = []
                    for m in range(2):
                        ap_ps = psAp.tile([128, NBLK], f32, name="ap")
                        for o in (0, 64):
                            hsl = slice(o, o + 64)
                            r = 32 * (2 * m + o // 64)
                            rsl = slice(r, r + 1)
                            for t in range(4 * b + 4):
                                s_ps = psSp.tile([128, NBLK], f32,
                                                 name="s_ps", tag="smm")
                                nc.tensor.matmul(
                                    s_ps[:],
                                    kT[m][hsl, t * 128:(t + 1) * 128],
                                    qT[m][hsl, bsl],
                                    start=True, stop=True,
                                    tile_position=(o, 0))
                                w_t = attwp.tile([128, NBLK], bf16,
                                                 name="w_t", bufs=3)
                                nc.scalar.activation(
                                    w_t[:], s_ps[:], AF.Exp, scale=0.125)
                                if t >= 4 * b:
                                    nc.vector.tensor_mul(
                                        w_t[:], w_t[:],
                                        masks_t[:, t - 4 * b, :])
                                nc.tensor.matmul(
                                    ap_ps[hsl, :],
                                    v_mem[:, t, m * 128 + o:
                                          m * 128 + o + 64],
                                    w_t[:],
                                    start=(t == 0), stop=(t == 4 * b + 3),
                                    tile_position=(0, o))
                                nc.tensor.matmul(
                                    den4[rsl, :], ones_c[:, 0:1], w_t[:],
                                    start=(t == 0), stop=(t == 4 * b + 3),
                                    tile_position=(0, r))
                        av_sb = attwp.tile([128, NBLK], bf16,
                                           name=f"av_sb{m}", bufs=1)
                        nc.vector.tensor_copy(av_sb[:], ap_ps[:])
                        avs.append(av_sb)
                    # widened self/denominator chain (rows {0,32,64,96}
                    # meaningful, other rows harmless garbage)
                    swf = attrp.tile([128, NBLK], f32, name="swf")
                    nc.scalar.activation(swf[:], sf_all[:, bsl], AF.Exp,
                                         scale=0.125)
                    dent = attrp.tile([128, NBLK], f32, name="dent")
                    nc.vector.tensor_add(dent[:], den4[:], swf[:])
                    rcp = attrp.tile([128, NBLK], f32, name="rcp")
                    nc.vector.reciprocal(rcp[:], dent[:])
                    swb = attrp.tile([128, NBLK], bf16, name="swb")
                    nc.vector.tensor_copy(swb[:], swf[:])
                    rcpb = attrp.tile([128, NBLK], bf16, name="rcpb")
                    nc.vector.tensor_copy(rcpb[:], rcp[:])
                    cmbs = []
                    for m in range(2):
                        sb_ps = psBrp.tile([128, NBLK], f32, name="br",
                                           tag="br")
                        for o in (0, 64):
                            r = 32 * (2 * m + o // 64)
                            rsl = slice(r, r + 1)
                            nc.tensor.matmul(
                                sb_ps[o:o + 64, :], ones128[rsl, 0:64],
                                swb[rsl, :], start=True, stop=True,
                                tile_position=(r, o))
                        t0 = atttp.tile([128, NBLK], bf16, name="t0",
                                        bufs=2)
                        nc.vector.tensor_mul(t0[:], vTh[m][:, bsl],
                                             sb_ps[:])
                        rb_ps = psBrp.tile([128, NBLK], f32, name="br",
                                           tag="br")
                        for o in (0, 64):
                            r = 32 * (2 * m + o // 64)
                            rsl = slice(r, r + 1)
                            nc.tensor.matmul(
                                rb_ps[o:o + 64, :], ones128[rsl, 0:64],
                                rcpb[rsl, :], start=True, stop=True,
                                tile_position=(r, o))
                        t1 = atttp.tile([128, NBLK], bf16, name="t1",
                                        bufs=2)
                        nc.vector.tensor_add(t1[:], avs[m][:], t0[:])
                        cmb = atttp.tile([128, NBLK], bf16, name=f"cmb{m}",
                                         bufs=1)
                        nc.vector.tensor_mul(cmb[:], t1[:], rb_ps[:])
                        cmbs.append(cmb)
                    # local o_proj: out = full D, contraction over local J;
                    # fold hidden/8 so the AllReduce yields h directly
                    for md in range(KC):
                        o_ps = psSp.tile([128, NBLK], f32, name="o_ps",
                                         tag="smm")
                        nc.tensor.matmul(
                            o_ps[:], woT_t[:, 0, md * 128:(md + 1) * 128],
                            cmbs[0][:], start=True, stop=False)
                        nc.tensor.matmul(
                            o_ps[:], woT_t[:, 1, md * 128:(md + 1) * 128],
                            cmbs[1][:], start=False, stop=True)
                        xhb = xhbp.tile([128, NBLK], bf16, name="xhb")
                        nc.sync.dma_start(xhb[:], xh_e[:, md, bsl])
                        oc = atttp.tile([128, NBLK], bf16, name="oc",
                                        bufs=2)
                        nc.vector.scalar_tensor_tensor(
                            out=oc[:], in0=xhb[:], scalar=0.125,
                            in1=o_ps[:], op0=Alu.mult, op1=Alu.add)
                        nc.scalar.dma_start(
                            o_bnc[b][md * 128:(md + 1) * 128, :], oc[:])
                    nc.gpsimd.collective_compute(
                        "AllReduce", Alu.add, replica_groups=rg,
                        ins=[o_bnc[b].opt()], outs=[h_sh[b].opt()])

                def emit_down(bb, md, hb_t, m_loc, pool, tag):
                    d_ps = pool.tile([128, NBLK], f32, name="d_ps",
                                     tag=tag)
                    for fc in range(FC):
                        nc.tensor.matmul(
                            d_ps[:], wdT_t[:, fc, md * 128:(md + 1) * 128],
                            m_loc[:, fc, :],
                            start=(fc == 0), stop=(fc == FC - 1))
                    hd = hdp.tile([128, NBLK], bf16, name="hd")
                    nc.sync.dma_start(
                        hd[:],
                        h_sh[bb][md * 128:(md + 1) * 128, :])
                    db = gutp.tile([128, NBLK], bf16, name="db")
                    nc.vector.scalar_tensor_tensor(
                        out=db[:], in0=hd[:], scalar=0.125,
                        in1=d_ps[:], op0=Alu.mult, op1=Alu.add)
                    nc.scalar.dma_start(
                        d_bnc[bb][md * 128:(md + 1) * 128, :], db[:])

                mloc_tiles = {}
                hb_tiles = {}

                def mlp_block(b):
                    bsl = slice(b * NBLK, (b + 1) * NBLK)
                    hb_t = hbp.tile([128, KC, NBLK], bf16, name="hb",
                                    bufs=1)
                    nc.sync.dma_start(
                        hb_t[:],
                        h_sh[b][:].rearrange("(t p) s -> p t s", p=128))
                    hb_tiles[b] = hb_t
                    # LN2 row stats: sum at row 0, sumsq at row 32 of one
                    # PSUM bank (sequential accumulation groups)
                    st_ps = psD2p.tile([128, NBLK], f32, name="d_ps",
                                       tag="dst")
                    for kc in range(KC):
                        nc.tensor.matmul(
                            st_ps[0:1, :], ones_c[:, 0:1], hb_t[:, kc, :],
                            start=(kc == 0), stop=(kc == KC - 1),
                            tile_position=(0, 0))
                    for kc in range(KC):
                        sq_t = sq2p.tile([128, NBLK], bf16, name="sq2_t")
                        nc.vector.tensor_mul(sq_t[:], hb_t[:, kc, :],
                                             hb_t[:, kc, :])
                        nc.tensor.matmul(
                            st_ps[32:33, :], ones_c[:, 0:1], sq_t[:],
                            start=(kc == 0), stop=(kc == KC - 1),
                            tile_position=(0, 32))
                    # broadcast raw sums, then widened stats math
                    srow = sm2p.tile([1, NBLK], bf16, name="srow")
                    nc.scalar.activation(srow[:], st_ps[0:1, :], AF.Copy,
                                         scale=1.0 / D)
                    qrow = sm2p.tile([1, NBLK], bf16, name="qrow")
                    nc.scalar.activation(qrow[:], st_ps[32:33, :], AF.Copy,
                                         scale=1.0 / D)
                    mean_ps = psGp.tile([128, NBLK], f32, name="g_ps",
                                        tag="g")
                    nc.tensor.matmul(mean_ps[:], ones128[0:1, :], srow[:],
                                     start=True, stop=True)
                    ex2_ps = psUp.tile([128, NBLK], f32, name="u_ps",
                                       tag="u")
                    nc.tensor.matmul(ex2_ps[:], ones128[0:1, :], qrow[:],
                                     start=True, stop=True)
                    # tmp: mean^2 -> var -> sd -> 1/sd (aliased in place)
                    tmp_t = sm2p.tile([128, NBLK], f32, name="tmp")
                    nc.scalar.activation(tmp_t[:], mean_ps[:], AF.Square)
                    nc.vector.tensor_sub(tmp_t[:], ex2_ps[:], tmp_t[:])
                    nc.scalar.activation(tmp_t[:], tmp_t[:], AF.Sqrt,
                                         bias=eps_c[:])
                    nc.vector.reciprocal(tmp_t[:], tmp_t[:])
                    rstd_t = sm2p.tile([128, NBLK], bf16, name="rstd")
                    nc.vector.tensor_copy(rstd_t[:], tmp_t[:])
                    c2_t = sm2p.tile([128, NBLK], bf16, name="c2")
                    nc.vector.tensor_mul(c2_t[:], mean_ps[:], tmp_t[:])
                    h2_t = h2p.tile([128, KC, NBLK], bf16, name="h2",
                                    bufs=1)
                    for kc in range(KC):
                        nc.vector.tensor_mul(h2_t[:, kc, :], hb_t[:, kc, :],
                                             rstd_t[:])
                        nc.vector.tensor_sub(h2_t[:, kc, :], h2_t[:, kc, :],
                                             c2_t[:])
                    # g/u with streamed weights (quarters: 2 mf each)
                    m_loc = mlocp.tile([128, MF, NBLK], bf16, name="m_loc")
                    mloc_tiles[b] = m_loc
                    for mf in range(MF):
                        if mf % 2 == 0:
                            wgq = mlpwp.tile([128, KC, 256], bf16,
                                             name="wgq")
                            nc.sync.dma_start(
                                wgq[:], wg_e[:, :, mf * 128:(mf + 2) * 128])
                            wuq = mlpwp.tile([128, KC, 256], bf16,
                                             name="wuq")
                            nc.scalar.dma_start(
                                wuq[:], wu_e[:, :, mf * 128:(mf + 2) * 128])
                        wofs = (mf % 2) * 128
                        psg = psGp.tile([128, NBLK], f32, name="g_ps",
                                        tag="g")
                        for kc in range(KC):
                            nc.tensor.matmul(
                                psg[:], wgq[:, kc, wofs:wofs + 128],
                                h2_t[:, kc, :],
                                start=(kc == 0), stop=(kc == KC - 1))
                        psu = psUp.tile([128, NBLK], f32, name="u_ps",
                                        tag="u")
                        for kc in range(KC):
                            nc.tensor.matmul(
                                psu[:], wuq[:, kc, wofs:wofs + 128],
                                h2_t[:, kc, :],
                                start=(kc == 0), stop=(kc == KC - 1))
                        sg = gutp.tile([128, NBLK], bf16, name="sg")
                        nc.scalar.activation(sg[:], psg[:], AF.Silu,
                                             bias=bg_t[:, mf:mf + 1])
                        nc.vector.scalar_tensor_tensor(
                            out=m_loc[:, mf, :], in0=psu[:],
                            scalar=bu_t[:, mf:mf + 1], in1=sg[:],
                            op0=Alu.add, op1=Alu.mult)
                        # interleave previous block's down proj (2 per mf)
                        if b >= 1:
                            for md in (2 * mf, 2 * mf + 1):
                                emit_down(b - 1, md, hb_tiles[b - 1],
                                          mloc_tiles[b - 1], psD2p, "dst")

                for b in range(NB):
                    attention_block(b)
                for b in range(NB):
                    mlp_block(b)
                    if b >= 1:
                        # down of b-1 completed inside mlp_block(b)
                        nc.gpsimd.collective_compute(
                            "ReduceScatter", Alu.add, replica_groups=rg,
                            ins=[d_bnc[b - 1].opt()],
                            outs=[fin[b - 1].opt()])
                # final block's down proj (alternate psum pools to avoid
                # drain stalls)
                for md in range(KC):
                    pool, tag = ((psD2p, "dst") if md % 2 == 0
                                 else (psGp, "g"))
                    emit_down(NB - 1, md, hb_tiles[NB - 1],
                              mloc_tiles[NB - 1], pool, tag)
                nc.gpsimd.collective_compute(
                    "ReduceScatter", Alu.add, replica_groups=rg,
                    ins=[d_bnc[NB - 1].opt()], outs=[fin[NB - 1].opt()])
                for b in range(NB):
                    nc.sync.dma_start(
                        out_e[:, b * NBLK:(b + 1) * NBLK], fin[b][:])
                if DEBUG:
                    for b in range(NB):
                        nc.sync.dma_start(
                            hdbg_e[:, b * NBLK:(b + 1) * NBLK],
                            h_sh[b][:])
            for p_ in reversed((wmatp, attwp, atttp, attrp, xhbp, mlpwp,
                                hbp, hdp, h2p, mlocp, gutp, sq2p, sm2p,
                                psSp, psAp, psDnp, psBrp, psGp, psUp,
                                psD2p)):
                p_.release()
            statkp.release()
            kqvp.release()

    return nc


# ---------------------------------------------------------------------------
# Host side
# ---------------------------------------------------------------------------

def _chunkT(a):
    """[R, D] -> [128, D//128, R] view for lhsT/rhs chunk layout.

    Result[p, kc, r] = a[r, kc*128 + p].
    """
    R, Dd = a.shape
    return np.ascontiguousarray(
        a.reshape(R, Dd // 128, 128).transpose(2, 1, 0))


def prepare_inputs(hidden_states, memory, position_ids,
                   ln1_w, ln1_b, ln2_w, ln2_b,
                   Wq, Wk, Wv, Wo, Wg, Wu, Wd, S):
    """Build the 8 per-core in_maps (numpy host prep)."""
    f32 = np.float32
    hid = np.asarray(hidden_states, f32)[0]       # [S, D]
    mem = np.asarray(memory, f32)[0]
    pos = np.asarray(position_ids)[0].astype(np.float64)

    Wq1 = np.asarray(Wq, f32) * np.asarray(ln1_w, f32)[None, :]
    Wk1 = np.asarray(Wk, f32) * np.asarray(ln1_w, f32)[None, :]
    Wv1 = np.asarray(Wv, f32) * np.asarray(ln1_w, f32)[None, :]
    bq = np.asarray(Wq, f32) @ np.asarray(ln1_b, f32)
    bk = np.asarray(Wk, f32) @ np.asarray(ln1_b, f32)
    bv = np.asarray(Wv, f32) @ np.asarray(ln1_b, f32)
    Wg2 = np.asarray(Wg, f32) * np.asarray(ln2_w, f32)[None, :]
    Wu2 = np.asarray(Wu, f32) * np.asarray(ln2_w, f32)[None, :]
    bg = np.asarray(Wg, f32) @ np.asarray(ln2_b, f32)
    bu = np.asarray(Wu, f32) @ np.asarray(ln2_b, f32)
    Wo_ = np.asarray(Wo, f32)
    Wd_ = np.asarray(Wd, f32)

    # x^T chunk layouts (shared by all cores)
    xm = _chunkT(mem).astype(BF16)                # [128, KC, S]
    xh = _chunkT(hid).astype(BF16)

    # rope tables [128, S], row pattern period 16
    inv = BASE ** (-(np.arange(8, dtype=np.float64) * 2) / RD)
    t = pos[:, None] * inv[None, :]               # [S, 8]
    cos8 = np.cos(t).T                            # [8, S]
    sin8 = np.sin(t).T
    cos16 = np.concatenate([cos8, cos8], 0)       # [16, S]
    sin16 = np.concatenate([-sin8, sin8], 0)
    cosf = np.tile(cos16, (8, 1)).astype(BF16)    # [128, S]
    sinf = np.tile(sin16, (8, 1)).astype(BF16)

    # strict-causal masks for the 4 diagonal-band offsets
    ii = np.arange(128)[:, None]
    jj = np.arange(NBLK)[None, :]
    masks = np.stack(
        [(ii + 128 * o < jj) for o in range(4)], 1).astype(BF16)

    in_maps = []
    for c in range(N_CORES):
        jsl = slice(c * J, (c + 1) * J)
        fsl = slice(c * FFL, (c + 1) * FFL)
        wq_c = Wq1[jsl]                            # [J, D]
        wk_c = Wk1[jsl]
        wv_c = Wv1[jsl]
        im = {
            "xm": xm, "xh": xh,
            "wq": _chunkT(wq_c).astype(BF16),
            "wk": _chunkT(wk_c).astype(BF16),
            "wv": _chunkT(wv_c).astype(BF16),
            "woT": _chunkT(Wo_[:, jsl]).astype(BF16),
            "wg": _chunkT(Wg2[fsl]).astype(BF16),
            "wu": _chunkT(Wu2[fsl]).astype(BF16),
            "wdT": _chunkT(Wd_[:, fsl]).astype(BF16),
            "wsq": np.ascontiguousarray(
                wq_c.sum(1).reshape(MD, 128).T).astype(f32),
            "wsk": np.ascontiguousarray(
                wk_c.sum(1).reshape(MD, 128).T).astype(f32),
            "wsvc": np.ascontiguousarray(
                wv_c.sum(1).reshape(MD, 128).T).astype(f32),
            "bq": np.ascontiguousarray(
                bq[jsl].reshape(MD, 128).T).astype(f32),
            "bk": np.ascontiguousarray(
                bk[jsl].reshape(MD, 128).T).astype(f32),
            "bvc": np.ascontiguousarray(
                bv[jsl].reshape(MD, 128).T).astype(f32),
            "wsv_row": wv_c.sum(1)[None, :].astype(f32),
            "bv_row": bv[jsl][None, :].astype(f32),
            "bg": np.ascontiguousarray(
                bg[fsl].reshape(MF, 128).T).astype(f32),
            "bu": np.ascontiguousarray(
                bu[fsl].reshape(MF, 128).T).astype(f32),
            "rope_cos": cosf, "rope_sinsg": sinf,
            "masks": masks,
        }
        in_maps.append(im)
    return in_maps


def assemble_output(results, S):
    outT = np.concatenate(
        [np.asarray(results[c]["out"]).astype(np.float32)
         for c in range(N_CORES)], 0)              # [D, S]
    return np.ascontiguousarray(outT.T).reshape(1, S, D).astype(np.float32)


_GRAPH_CACHE = {}


def get_graph(S):
    if S not in _GRAPH_CACHE:
        _GRAPH_CACHE[S] = build_graph(S)
    return _GRAPH_CACHE[S]


def kernel(hidden_states, memory, attention_mask, position_ids,
           ln1_w, ln1_b, ln2_w, ln2_b, Wq, Wk, Wv, Wo, Wg, Wu, Wd):
    from concourse.bass_utils import run_bass_kernel_spmd

    S = np.asarray(hidden_states).shape[1]
    in_maps = prepare_inputs(
        hidden_states, memory, position_ids, ln1_w, ln1_b, ln2_w, ln2_b,
        Wq, Wk, Wv, Wo, Wg, Wu, Wd, S)
    nc = get_graph(S)
    res = run_bass_kernel_spmd(nc, in_maps, core_ids=list(range(N_CORES)))
    return assemble_output(res.results, S)
